# revision 1
# baseline (speedup 1.0000x reference)
"""FFT-based 2D long convolution on 8 Trainium2 NeuronCores.

Reference op (per (b,c) plane, 512x512 FFT):
    y = irfft2(rfft2(x, s=(512,512)) * rfft2(filt[c], s=(512,512)),
               s=(512,512), norm="forward")[..., :256, :256] + x

DFTs as dense matmuls on the tensor engine, with the *data* always the
stationary operand (out = lhsT.T @ rhs flips the data layout each stage), so
the 4 contractions chain with zero transposes:

    s1: T[w,hf]  = sum_h  x[h,w]  * Ah[h,hf]        x:[H,W]   -> T:[W,HF]
    s2: S[hf,wf] = sum_w  T[w,hf] * Aw[w,wf]        T:[W,HF]  -> S:[HF,WF]
    oK: P = S * K[c]   (pointwise complex, DVE, fused with PSUM->SBUF)
    s3: V[wf,h]  = sum_hf P[hf,wf]* Bh[hf,h]        P:[HF,WF] -> V:[WF,H]
    s4: y[h,w]   = sum_wf Vre*Gc - Vim*Gs           V:[WF,H]  -> y:[H,W]
    y += x

Sharding: channels across the 8 cores (8 ch/core x 8 batch = 64 planes/core);
filter spectra K[c] computed once per core, cached in SBUF. All matmuls are
float32r (full-rate fp32, free dim >= 256).

TRN2 constraint: a fused fp32r matmul (S3_LW) can carry at most ONE sem wait.
Structure guarantees <=1 cross-engine dep per matmul:
  - per-stage PSUM pools so each slot's releasing engine is deterministic
    (s1: DVE T-copies; s2/s4 shared pool: DVE oK/residual; s3+nyq: ACT V-copies)
  - tiny "touch" matmuls absorb the DMA / producer dep into PE program order
    before each stage's first real matmul.
"""

import numpy as np
from contextlib import ExitStack

import concourse.bass as bass
import concourse.mybir as mybir
import concourse.tile as tile
from concourse.bass_utils import run_bass_kernel_spmd

B, C, H, W = 8, 64, 256, 256
N = 512
HF = 512
WF = 257
WFP = 258          # fp32r matmul needs even moving free size
NCORES = 8
CPC = C // NCORES
PLANES = CPC * B

F32 = mybir.dt.float32
F32R = mybir.dt.float32r


def _consts():
    h = np.arange(H, dtype=np.float64)[:, None]
    hf = np.arange(HF, dtype=np.float64)[None, :]
    ah = np.exp(-2j * np.pi * h * hf / N)              # [256, 512]
    w = np.arange(W, dtype=np.float64)[:, None]
    wf = np.arange(WF, dtype=np.float64)[None, :]
    aw = np.exp(-2j * np.pi * w * wf / N)              # [256, 257]
    aw = np.concatenate([aw, np.zeros((W, 1))], axis=1)  # pad to 258 (even N)
    hf2 = np.arange(HF, dtype=np.float64)[:, None]
    h2 = np.arange(H, dtype=np.float64)[None, :]
    bh = np.exp(+2j * np.pi * hf2 * h2 / N)            # [512, 256]
    c = np.full((WF, 1), 2.0); c[0] = 1.0; c[256] = 1.0
    wf2 = np.arange(WF, dtype=np.float64)[:, None]
    w2 = np.arange(W, dtype=np.float64)[None, :]
    gc = c * np.cos(2 * np.pi * wf2 * w2 / N)          # [257, 256]
    gs = c * np.sin(2 * np.pi * wf2 * w2 / N)          # [257, 256]
    f = np.float32
    d = {
        "ahr": (f(ah.real), 2), "ahi": (f(ah.imag), 2),
        "awr": (f(aw.real), 2), "awi": (f(aw.imag), 2), "awin": (f(-aw.imag), 2),
        "bhr": (f(bh.real), 4), "bhi": (f(bh.imag), 4), "bhin": (f(-bh.imag), 4),
        "gc": (f(gc[:256]), 2), "gsn": (f(-gs[:256]), 2),
    }
    # one [128, F] blob in SBUF layout -> one DMA, one semaphore
    cols, offs, off = [], {}, 0
    for k, (arr, kt) in d.items():
        fd = arr.shape[1]
        cols.append(arr.reshape(kt, 128, fd).transpose(1, 0, 2).reshape(128, kt * fd))
        offs[k] = (off, fd)
        off += kt * fd
    pm1 = np.zeros((128, W), np.float32)
    pm1[0] = f(gc[256])
    cols.append(pm1)
    offs["pm1"] = (off, W)
    return np.concatenate(cols, axis=1), offs


def _legalize_waits(nc, max_waits=1):
    """This walrus build allows only ONE sem wait per engine instruction
    ("Too many sync wait commands"). Split extra waits onto same-engine NOPs
    inserted immediately before — engine program order preserves semantics."""
    k = 0
    for fn in nc.m.functions:
        for bb in fn.blocks:
            new = []
            for ins in bb.instructions:
                si = ins.sync_info
                waits = list(si.on_wait) if (si and si.on_wait) else []
                if len(waits) > max_waits:
                    for w in waits[:-max_waits]:
                        k += 1
                        new.append(mybir.InstNoOp(
                            name=f"{ins.name}-lw{k}", engine=ins.engine,
                            ins=[], outs=[],
                            sync_info=mybir.SyncInfo(on_wait=[w], on_update=[])))
                    ins.sync_info = mybir.SyncInfo(
                        on_wait=waits[-max_waits:],
                        on_update=list(si.on_update or []))
                new.append(ins)
            bb.instructions = new
    return k


def build_nc(n_ch=CPC, n_b=B, reps=1):
    nc = bass.Bass(trn_type="TRN2")
    n_planes = n_ch * n_b

    xs = nc.dram_tensor("xs", [n_planes, H, W], F32R, kind="ExternalInput").ap()
    fs = nc.dram_tensor("fs", [n_ch, H, W], F32R, kind="ExternalInput").ap()
    cblob_np, coffs = _consts()
    cb_d = nc.dram_tensor("cblob", list(cblob_np.shape), F32R,
                          kind="ExternalInput").ap()
    ys = nc.dram_tensor("ys", [n_planes, H, W], F32, kind="ExternalOutput").ap()

    with tile.TileContext(nc) as tc, ExitStack() as ctx:
        const_p = ctx.enter_context(tc.tile_pool(name="const", bufs=1))
        kc_p = ctx.enter_context(tc.tile_pool(name="kc", bufs=1))
        x_p = ctx.enter_context(tc.tile_pool(name="xp", bufs=3))
        t_p = ctx.enter_context(tc.tile_pool(name="tp", bufs=2))
        p_p = ctx.enter_context(tc.tile_pool(name="pp", bufs=2))
        v_p = ctx.enter_context(tc.tile_pool(name="vp", bufs=2))
        y_p = ctx.enter_context(tc.tile_pool(name="yp", bufs=2))
        tm_p = ctx.enter_context(tc.tile_pool(name="tm", bufs=8))
        ps1_p = ctx.enter_context(tc.tile_pool(name="ps1", bufs=2, space="PSUM"))
        psd_p = ctx.enter_context(tc.tile_pool(name="psd", bufs=3, space="PSUM"))
        ps3_p = ctx.enter_context(tc.tile_pool(name="ps3", bufs=2, space="PSUM"))
        dps_p = ctx.enter_context(tc.tile_pool(name="dps", bufs=1, space="PSUM"))

        cb = const_p.tile(list(cblob_np.shape), F32R, tag="cb")
        nc.sync.dma_start(out=cb, in_=cb_d)

        class CV:
            def __init__(self, name, fd):
                self.off, self.fd = coffs[name][0], fd
            def __getitem__(self, idx):
                p, k, fs_ = idx
                lo = self.off + k * self.fd
                if fs_ == slice(None):
                    return cb[p, lo:lo + self.fd]
                return cb[p, lo + fs_.start:lo + fs_.stop]

        ahr = CV("ahr", HF); ahi = CV("ahi", HF)
        awr = CV("awr", WFP); awi = CV("awi", WFP); awin = CV("awin", WFP)
        bhr = CV("bhr", H); bhi = CV("bhi", H); bhin = CV("bhin", H)
        gc = CV("gc", W); gsn = CV("gsn", W)
        pm1 = cb[0:1, coffs["pm1"][0]:coffs["pm1"][0] + W]

        kre = kc_p.tile([128, n_ch, 4, WFP], F32R, tag="kre")
        kim = kc_p.tile([128, n_ch, 4, WFP], F32R, tag="kim")

        MM = nc.tensor.matmul
        # single dummy PSUM target for all "touch" matmuls (PE-only WAW)
        dps = dps_p.tile([1, 64], F32, tag="dps")

        def touch(src_ap):
            """Tiny matmul reading src so PE inherits its producer dep."""
            MM(dps, src_ap[0:1, 0:1], src_ap[0:1, 0:64], start=True, stop=True)

        # PE touches the const blob once; const deps then PE-dominated.
        touch(cb)

        def fwd(plane_ap, sink):
            """s1+s2 on one [256,256] DRAM plane; sink(mhf, sr, si) consumes
            the four [128,WF] PSUM spectrum chunk pairs. Returns x tile."""
            xt = x_p.tile([128, 2, W], F32R, tag="xt")
            nc.sync.dma_start(out=xt, in_=plane_ap.rearrange("(k p) w -> p k w", p=128))
            touch(xt[:, 0, :])          # absorb DMA wait
            tre = t_p.tile([128, 2, HF], F32R, tag="tre")
            tim = t_p.tile([128, 2, HF], F32R, tag="tim")
            for mw in range(2):
                pr = ps1_p.tile([128, HF], F32, tag="ps1")
                pi = ps1_p.tile([128, HF], F32, tag="ps1")
                for kh in range(2):
                    lhsT = xt[:, kh, mw * 128:(mw + 1) * 128]
                    MM(pr, lhsT, ahr[:, kh, :], start=(kh == 0), stop=(kh == 1))
                    MM(pi, lhsT, ahi[:, kh, :], start=(kh == 0), stop=(kh == 1))
                nc.vector.tensor_copy(tre[:, mw, :], pr)
                nc.vector.tensor_copy(tim[:, mw, :], pi)
            for mhf in range(4):
                sr = psd_p.tile([128, WFP], F32, tag="psd")
                si = psd_p.tile([128, WFP], F32, tag="psd")
                for kw in range(2):
                    lre = tre[:, kw, mhf * 128:(mhf + 1) * 128]
                    lim = tim[:, kw, mhf * 128:(mhf + 1) * 128]
                    MM(sr, lre, awr[:, kw, :], start=(kw == 0), stop=False)
                    MM(sr, lim, awin[:, kw, :], start=False, stop=(kw == 1))
                    MM(si, lre, awi[:, kw, :], start=(kw == 0), stop=False)
                    MM(si, lim, awr[:, kw, :], start=False, stop=(kw == 1))
                sink(mhf, sr, si)
            return xt

        # ---- filter spectra into K cache (DVE copies keep psd DVE-released)
        for ch in range(n_ch):
            def k_sink(mhf, sr, si, ch=ch):
                nc.vector.tensor_copy(kre[:, ch, mhf, :], sr)
                nc.vector.tensor_copy(kim[:, ch, mhf, :], si)
            fwd(fs[ch], k_sink)

        # ---- main plane loop (optionally repeated on-device for timing) ----
        rep_ctx = tc.For_i(0, reps, 1) if reps > 1 else None
        if rep_ctx is not None:
            rep_ctx.__enter__()
        for ch in range(n_ch):
            for b in range(n_b):
                pl = ch * n_b + b
                pre = p_p.tile([128, 4, WFP], F32R, tag="pre")
                pim = p_p.tile([128, 4, WFP], F32R, tag="pim")

                def x_sink(mhf, sr, si, ch=ch, pre=pre, pim=pim):
                    krc = kre[:, ch, mhf, :]
                    kic = kim[:, ch, mhf, :]
                    t1 = tm_p.tile([128, WFP], F32, tag="tm")
                    t2 = tm_p.tile([128, WFP], F32, tag="tm")
                    t3 = tm_p.tile([128, WFP], F32, tag="tm")
                    t4 = tm_p.tile([128, WFP], F32, tag="tm")
                    nc.vector.tensor_mul(t1, sr, krc)
                    nc.vector.tensor_mul(t2, si, kic)
                    nc.vector.tensor_sub(pre[:, mhf, :], t1, t2)
                    nc.vector.tensor_mul(t3, sr, kic)
                    nc.vector.tensor_mul(t4, si, krc)
                    nc.vector.tensor_add(pim[:, mhf, :], t3, t4)

                xt = fwd(xs[pl], x_sink)

                touch(pre[:, 0, :])     # absorb DVE oK dep before s3
                vre = v_p.tile([128, 2, H], F32R, tag="vre")
                vim = v_p.tile([128, 2, H], F32R, tag="vim")
                vnyq = v_p.tile([1, H], F32R, tag="vnyq")
                for mwf in range(2):
                    pvr = ps3_p.tile([128, H], F32, tag="ps3")
                    pvi = ps3_p.tile([128, H], F32, tag="ps3")
                    for khf in range(4):
                        lre = pre[:, khf, mwf * 128:(mwf + 1) * 128]
                        lim = pim[:, khf, mwf * 128:(mwf + 1) * 128]
                        MM(pvr, lre, bhr[:, khf, :], start=(khf == 0), stop=False)
                        MM(pvr, lim, bhin[:, khf, :], start=False, stop=(khf == 3))
                        MM(pvi, lre, bhi[:, khf, :], start=(khf == 0), stop=False)
                        MM(pvi, lim, bhr[:, khf, :], start=False, stop=(khf == 3))
                    nc.scalar.copy(out=vre[:, mwf, :], in_=pvr)
                    nc.scalar.copy(out=vim[:, mwf, :], in_=pvi)
                pvn = ps3_p.tile([1, H], F32, tag="ps3")
                for khf in range(4):
                    MM(pvn, pre[:, khf, 256:257], bhr[:, khf, :],
                       start=(khf == 0), stop=False)
                    MM(pvn, pim[:, khf, 256:257], bhin[:, khf, :],
                       start=False, stop=(khf == 3))
                nc.scalar.copy(out=vnyq, in_=pvn)

                touch(vre[:, 0, :])     # absorb ACT V-copy dep before s4
                ysb = y_p.tile([128, 2, W], F32, tag="ysb")
                for mh in range(2):
                    py = psd_p.tile([128, W], F32, tag="psd")
                    MM(py, vre[:, 0, mh * 128:(mh + 1) * 128], gc[:, 0, :],
                       start=True, stop=False)
                    MM(py, vim[:, 0, mh * 128:(mh + 1) * 128], gsn[:, 0, :],
                       start=False, stop=False)
                    MM(py, vre[:, 1, mh * 128:(mh + 1) * 128], gc[:, 1, :],
                       start=False, stop=False)
                    MM(py, vim[:, 1, mh * 128:(mh + 1) * 128], gsn[:, 1, :],
                       start=False, stop=False)
                    MM(py, vnyq[0:1, mh * 128:(mh + 1) * 128], pm1,
                       start=False, stop=True)
                    nc.vector.tensor_add(ysb[:, mh, :], py, xt[:, mh, :])
                nc.sync.dma_start(out=ys[pl].rearrange("(k p) w -> p k w", p=128),
                                  in_=ysb)
        if rep_ctx is not None:
            rep_ctx.__exit__(None, None, None)
    _legalize_waits(nc)
    return nc


def kernel(x: np.ndarray, filt: np.ndarray) -> np.ndarray:
    x = np.ascontiguousarray(x, dtype=np.float32)
    filt = np.ascontiguousarray(filt, dtype=np.float32)
    cblob = _consts()[0]
    nc = build_nc()
    in_maps = []
    for i in range(NCORES):
        sl = slice(i * CPC, (i + 1) * CPC)
        xsh = np.ascontiguousarray(
            x[:, sl].transpose(1, 0, 2, 3).reshape(PLANES, H, W))
        in_maps.append({"xs": xsh, "fs": np.ascontiguousarray(filt[sl]),
                        "cblob": cblob})
    res = run_bass_kernel_spmd(nc, in_maps, core_ids=list(range(NCORES)))
    out = np.empty_like(x)
    for i in range(NCORES):
        sl = slice(i * CPC, (i + 1) * CPC)
        out[:, sl] = res.results[i]["ys"].reshape(CPC, B, H, W).transpose(1, 0, 2, 3)
    return out



# revision 12
# speedup vs baseline: 1.1201x; 1.1201x over previous
"""FFT-based 2D long convolution on 8 Trainium2 NeuronCores.

Reference op (per (b,c) plane, 512x512 FFT):
    y = irfft2(rfft2(x, s=(512,512)) * rfft2(filt[c], s=(512,512)),
               s=(512,512), norm="forward")[..., :256, :256] + x

DFTs as dense matmuls on the tensor engine, with the *data* always the
stationary operand (out = lhsT.T @ rhs flips the data layout each stage), so
the 4 contractions chain with zero transposes:

    s1: T[w,hf]  = sum_h  x[h,w]  * Ah[h,hf]        x:[H,W]   -> T:[W,HF]
    s2: S[hf,wf] = sum_w  T[w,hf] * Aw[w,wf]        T:[W,HF]  -> S:[HF,WF]
    oK: P = S * K[c]   (pointwise complex, DVE, fused with PSUM->SBUF)
    s3: V[wf,h]  = sum_hf P[hf,wf]* Bh[hf,h]        P:[HF,WF] -> V:[WF,H]
    s4: y[h,w]   = sum_wf Vre*Gc - Vim*Gs           V:[WF,H]  -> y:[H,W]
    y += x

Sharding: channels across the 8 cores (8 ch/core x 8 batch = 64 planes/core);
filter spectra K[c] computed once per core, cached in SBUF. All matmuls are
float32r (full-rate fp32, free dim >= 256).

TRN2 constraint: a fused fp32r matmul (S3_LW) can carry at most ONE sem wait.
Structure guarantees <=1 cross-engine dep per matmul:
  - per-stage PSUM pools so each slot's releasing engine is deterministic
    (s1: DVE T-copies; s2/s4 shared pool: DVE oK/residual; s3+nyq: ACT V-copies)
  - tiny "touch" matmuls absorb the DMA / producer dep into PE program order
    before each stage's first real matmul.
"""

import numpy as np
import ml_dtypes
from contextlib import ExitStack

import concourse.bass as bass
import concourse.mybir as mybir
import concourse.tile as tile
from concourse.bass_utils import run_bass_kernel_spmd

B, C, H, W = 8, 64, 256, 256
N = 512
HF = 512
WF = 257
WFP = 258          # even moving free size
NCORES = 8
CPC = C // NCORES
PLANES = CPC * B

F32 = mybir.dt.float32
F32R = mybir.dt.float32r
BF16 = mybir.dt.bfloat16
NPBF16 = ml_dtypes.bfloat16


def _consts():
    h = np.arange(H, dtype=np.float64)[:, None]
    hf = np.arange(HF, dtype=np.float64)[None, :]
    ah = np.exp(-2j * np.pi * h * hf / N)              # [256, 512]
    w = np.arange(W, dtype=np.float64)[:, None]
    wf = np.arange(WF, dtype=np.float64)[None, :]
    aw = np.exp(-2j * np.pi * w * wf / N)              # [256, 257]
    aw = np.concatenate([aw, np.zeros((W, 1))], axis=1)  # pad to 258 (even N)
    hf2 = np.arange(HF, dtype=np.float64)[:, None]
    h2 = np.arange(H, dtype=np.float64)[None, :]
    bh = np.exp(+2j * np.pi * hf2 * h2 / N)            # [512, 256]
    c = np.full((WF, 1), 2.0); c[0] = 1.0; c[256] = 1.0
    wf2 = np.arange(WF, dtype=np.float64)[:, None]
    w2 = np.arange(W, dtype=np.float64)[None, :]
    gc = c * np.cos(2 * np.pi * wf2 * w2 / N)          # [257, 256]
    gs = c * np.sin(2 * np.pi * wf2 * w2 / N)          # [257, 256]
    f = NPBF16
    d = {
        "ahr": (f(ah.real), 2), "ahi": (f(ah.imag), 2),
        "awr": (f(aw.real), 2), "awi": (f(aw.imag), 2), "awin": (f(-aw.imag), 2),
        "bhr": (f(bh.real), 4), "bhi": (f(bh.imag), 4), "bhin": (f(-bh.imag), 4),
        "gc": (f(gc[:256]), 2), "gsn": (f(-gs[:256]), 2),
    }
    # one [128, F] blob in SBUF layout -> one DMA, one semaphore
    cols, offs, off = [], {}, 0
    for k, (arr, kt) in d.items():
        fd = arr.shape[1]
        cols.append(arr.reshape(kt, 128, fd).transpose(1, 0, 2).reshape(128, kt * fd))
        offs[k] = (off, fd)
        off += kt * fd
    pm1 = np.zeros((128, W), NPBF16)
    pm1[0] = f(gc[256])
    cols.append(pm1)
    offs["pm1"] = (off, W)
    return np.concatenate(cols, axis=1), offs


def _legalize_waits(nc, max_waits=1):
    """This walrus build allows only ONE sem wait per engine instruction
    ("Too many sync wait commands"). Split extra waits onto same-engine NOPs
    inserted immediately before — engine program order preserves semantics."""
    k = 0
    for fn in nc.m.functions:
        for bb in fn.blocks:
            new = []
            for ins in bb.instructions:
                si = ins.sync_info
                waits = list(si.on_wait) if (si and si.on_wait) else []
                if len(waits) > max_waits:
                    for w in waits[:-max_waits]:
                        k += 1
                        new.append(mybir.InstNoOp(
                            name=f"{ins.name}-lw{k}", engine=ins.engine,
                            ins=[], outs=[],
                            sync_info=mybir.SyncInfo(on_wait=[w], on_update=[])))
                    ins.sync_info = mybir.SyncInfo(
                        on_wait=waits[-max_waits:],
                        on_update=list(si.on_update or []))
                new.append(ins)
            bb.instructions = new
    return k


def _dedupe_ldweights(nc):
    """Skip the PE stationary reload when consecutive matmuls in the final
    engine order share the identical weights AP (verified on HW: a matmul
    with ldweights=False reuses the array contents left by the previous
    self-loading matmul)."""
    def sig(ins):
        w = ins.ins[1]
        mr = w.memref
        return (mr.name if hasattr(mr, "name") else str(mr),
                w.offset, str(w.ap), str(w.dtype),
                ins.is_transpose, str(ins.perf_mode),
                tuple(ins.tile_position or ()), tuple(ins.tile_size or ()))
    n = 0
    for fn in nc.m.functions:
        for bb in fn.blocks:
            prev = None
            for ins in bb.instructions:
                if not isinstance(ins, mybir.InstMatmult):
                    continue
                s = sig(ins)
                if prev is not None and s == prev:
                    ins.ldweights = False
                    n += 1
                prev = s
    return n


def build_nc(n_ch=CPC, n_b=B, reps=1):
    nc = bass.Bass(trn_type="TRN2")
    n_planes = n_ch * n_b

    xs = nc.dram_tensor("xs", [n_planes, H, W], BF16, kind="ExternalInput").ap()
    fs = nc.dram_tensor("fs", [n_ch, H, W], BF16, kind="ExternalInput").ap()
    cblob_np, coffs = _consts()
    cb_d = nc.dram_tensor("cblob", list(cblob_np.shape), BF16,
                          kind="ExternalInput").ap()
    ys = nc.dram_tensor("ys", [n_planes, H, W], F32, kind="ExternalOutput").ap()

    with tile.TileContext(nc) as tc, ExitStack() as ctx:
        const_p = ctx.enter_context(tc.tile_pool(name="const", bufs=1))
        kc_p = ctx.enter_context(tc.tile_pool(name="kc", bufs=1))
        x_p = ctx.enter_context(tc.tile_pool(name="xp", bufs=3))
        t_p = ctx.enter_context(tc.tile_pool(name="tp", bufs=2))
        p_p = ctx.enter_context(tc.tile_pool(name="pp", bufs=2))
        v_p = ctx.enter_context(tc.tile_pool(name="vp", bufs=2))
        y_p = ctx.enter_context(tc.tile_pool(name="yp", bufs=2))
        tm_p = ctx.enter_context(tc.tile_pool(name="tm", bufs=8))
        ps1_p = ctx.enter_context(tc.tile_pool(name="ps1", bufs=2, space="PSUM"))
        psd_p = ctx.enter_context(tc.tile_pool(name="psd", bufs=3, space="PSUM"))
        ps3_p = ctx.enter_context(tc.tile_pool(name="ps3", bufs=2, space="PSUM"))
        dps_p = ctx.enter_context(tc.tile_pool(name="dps", bufs=1, space="PSUM"))

        cb = const_p.tile(list(cblob_np.shape), BF16, tag="cb")
        nc.sync.dma_start(out=cb, in_=cb_d)

        class CV:
            def __init__(self, name, fd):
                self.off, self.fd = coffs[name][0], fd
            def __getitem__(self, idx):
                p, k, fs_ = idx
                lo = self.off + k * self.fd
                if fs_ == slice(None):
                    return cb[p, lo:lo + self.fd]
                return cb[p, lo + fs_.start:lo + fs_.stop]

        ahr = CV("ahr", HF); ahi = CV("ahi", HF)
        awr = CV("awr", WFP); awi = CV("awi", WFP); awin = CV("awin", WFP)
        bhr = CV("bhr", H); bhi = CV("bhi", H); bhin = CV("bhin", H)
        gc = CV("gc", W); gsn = CV("gsn", W)
        pm1 = cb[0:1, coffs["pm1"][0]:coffs["pm1"][0] + W]

        kre = kc_p.tile([128, n_ch, 4, WFP], BF16, tag="kre")
        kim = kc_p.tile([128, n_ch, 4, WFP], BF16, tag="kim")

        MM = nc.tensor.matmul
        # single dummy PSUM target for all "touch" matmuls (PE-only WAW)
        dps = dps_p.tile([1, 64], F32, tag="dps")

        def touch(src_ap):
            """Tiny matmul reading src so PE inherits its producer dep."""
            MM(dps, src_ap[0:1, 0:1], src_ap[0:1, 0:64], start=True, stop=True)

        # PE touches the const blob once; const deps then PE-dominated.
        touch(cb)

        def fwd(plane_ap, sink):
            """s1+s2 on one [256,256] DRAM plane; sink(mhf, sr, si) consumes
            the four [128,WF] PSUM spectrum chunk pairs. Returns x tile."""
            xt = x_p.tile([128, 2, W], BF16, tag="xt")
            nc.sync.dma_start(out=xt, in_=plane_ap.rearrange("(k p) w -> p k w", p=128))
            touch(xt[:, 0, :])          # absorb DMA wait
            tre = t_p.tile([128, 2, HF], BF16, tag="tre")
            tim = t_p.tile([128, 2, HF], BF16, tag="tim")
            for mw in range(2):
                pr = ps1_p.tile([128, HF], F32, tag="ps1")
                pi = ps1_p.tile([128, HF], F32, tag="ps1")
                for kh in range(2):
                    lhsT = xt[:, kh, mw * 128:(mw + 1) * 128]
                    MM(pr, lhsT, ahr[:, kh, :], start=(kh == 0), stop=(kh == 1))
                    MM(pi, lhsT, ahi[:, kh, :], start=(kh == 0), stop=(kh == 1))
                nc.scalar.copy(out=tre[:, mw, :], in_=pr)
                nc.scalar.copy(out=tim[:, mw, :], in_=pi)
            for mhf in range(4):
                sr = psd_p.tile([128, WFP], F32, tag="psd")
                si = psd_p.tile([128, WFP], F32, tag="psd")
                for kw in range(2):
                    lre = tre[:, kw, mhf * 128:(mhf + 1) * 128]
                    lim = tim[:, kw, mhf * 128:(mhf + 1) * 128]
                    MM(sr, lre, awr[:, kw, :], start=(kw == 0), stop=False)
                    MM(si, lre, awi[:, kw, :], start=(kw == 0), stop=False)
                    MM(sr, lim, awin[:, kw, :], start=False, stop=(kw == 1))
                    MM(si, lim, awr[:, kw, :], start=False, stop=(kw == 1))
                sink(mhf, sr, si)
            return xt

        # ---- filter spectra into K cache (DVE copies keep psd DVE-released)
        for ch in range(n_ch):
            def k_sink(mhf, sr, si, ch=ch):
                nc.vector.tensor_copy(kre[:, ch, mhf, :], sr)
                nc.vector.tensor_copy(kim[:, ch, mhf, :], si)
            fwd(fs[ch], k_sink)

        # ---- main plane loop (optionally repeated on-device for timing) ----
        rep_ctx = tc.For_i(0, reps, 1) if reps > 1 else None
        if rep_ctx is not None:
            rep_ctx.__enter__()
        for ch in range(n_ch):
            for b in range(n_b):
                pl = ch * n_b + b
                pre = p_p.tile([128, 4, WFP], BF16, tag="pre")
                pim = p_p.tile([128, 4, WFP], BF16, tag="pim")

                def x_sink(mhf, sr, si, ch=ch, pre=pre, pim=pim):
                    krc = kre[:, ch, mhf, :]
                    kic = kim[:, ch, mhf, :]
                    t1 = tm_p.tile([128, WFP], F32, tag="tm")
                    t2 = tm_p.tile([128, WFP], F32, tag="tm")
                    t3 = tm_p.tile([128, WFP], F32, tag="tm")
                    t4 = tm_p.tile([128, WFP], F32, tag="tm")
                    nc.vector.tensor_mul(t1, sr, krc)
                    nc.vector.tensor_mul(t2, si, kic)
                    nc.vector.tensor_mul(t3, sr, kic)
                    nc.vector.tensor_mul(t4, si, krc)
                    nc.gpsimd.tensor_sub(pre[:, mhf, :], t1, t2)
                    nc.gpsimd.tensor_add(pim[:, mhf, :], t3, t4)

                xt = fwd(xs[pl], x_sink)

                touch(pre[:, 0, :])     # absorb gpsimd oK dep before s3
                vre = v_p.tile([128, 2, H], BF16, tag="vre")
                vim = v_p.tile([128, 2, H], BF16, tag="vim")
                vnyq = v_p.tile([1, H], BF16, tag="vnyq")
                for mwf in range(2):
                    pvr = ps3_p.tile([128, H], F32, tag="ps3")
                    pvi = ps3_p.tile([128, H], F32, tag="ps3")
                    for khf in range(4):
                        lre = pre[:, khf, mwf * 128:(mwf + 1) * 128]
                        lim = pim[:, khf, mwf * 128:(mwf + 1) * 128]
                        MM(pvr, lre, bhr[:, khf, :], start=(khf == 0), stop=False)
                        MM(pvi, lre, bhi[:, khf, :], start=(khf == 0), stop=False)
                        MM(pvr, lim, bhin[:, khf, :], start=False, stop=(khf == 3))
                        MM(pvi, lim, bhr[:, khf, :], start=False, stop=(khf == 3))
                    nc.scalar.copy(out=vre[:, mwf, :], in_=pvr)
                    nc.scalar.copy(out=vim[:, mwf, :], in_=pvi)
                pvn = ps3_p.tile([1, H], F32, tag="ps3")
                for khf in range(4):
                    MM(pvn, pre[:, khf, 256:257], bhr[:, khf, :],
                       start=(khf == 0), stop=False)
                    MM(pvn, pim[:, khf, 256:257], bhin[:, khf, :],
                       start=False, stop=(khf == 3))
                nc.scalar.copy(out=vnyq, in_=pvn)

                touch(vre[:, 0, :])     # absorb ACT V-copy dep before s4
                ysb = y_p.tile([128, 2, W], F32, tag="ysb")
                for mh in range(2):
                    py = psd_p.tile([128, W], F32, tag="psd")
                    MM(py, vre[:, 0, mh * 128:(mh + 1) * 128], gc[:, 0, :],
                       start=True, stop=False)
                    MM(py, vim[:, 0, mh * 128:(mh + 1) * 128], gsn[:, 0, :],
                       start=False, stop=False)
                    MM(py, vre[:, 1, mh * 128:(mh + 1) * 128], gc[:, 1, :],
                       start=False, stop=False)
                    MM(py, vim[:, 1, mh * 128:(mh + 1) * 128], gsn[:, 1, :],
                       start=False, stop=False)
                    MM(py, vnyq[0:1, mh * 128:(mh + 1) * 128], pm1,
                       start=False, stop=True)
                    nc.vector.tensor_add(ysb[:, mh, :], py, xt[:, mh, :])
                nc.sync.dma_start(out=ys[pl].rearrange("(k p) w -> p k w", p=128),
                                  in_=ysb)
        if rep_ctx is not None:
            rep_ctx.__exit__(None, None, None)
    _dedupe_ldweights(nc)
    _legalize_waits(nc)
    return nc


def kernel(x: np.ndarray, filt: np.ndarray) -> np.ndarray:
    x = np.ascontiguousarray(x, dtype=np.float32)
    xb = x.astype(NPBF16)
    fb = np.ascontiguousarray(filt, dtype=np.float32).astype(NPBF16)
    cblob = _consts()[0]
    nc = build_nc()
    in_maps = []
    for i in range(NCORES):
        sl = slice(i * CPC, (i + 1) * CPC)
        xsh = np.ascontiguousarray(
            xb[:, sl].transpose(1, 0, 2, 3).reshape(PLANES, H, W))
        in_maps.append({"xs": xsh, "fs": np.ascontiguousarray(fb[sl]),
                        "cblob": cblob})
    res = run_bass_kernel_spmd(nc, in_maps, core_ids=list(range(NCORES)))
    out = np.empty_like(x)
    for i in range(NCORES):
        sl = slice(i * CPC, (i + 1) * CPC)
        out[:, sl] = res.results[i]["ys"].reshape(CPC, B, H, W).transpose(1, 0, 2, 3)
    return out



# revision 20
# speedup vs baseline: 1.3302x; 1.1876x over previous
"""FFT-based 2D long convolution on 8 Trainium2 NeuronCores.

Reference op (per (b,c) plane, 512x512 FFT):
    y = irfft2(rfft2(x, s=(512,512)) * rfft2(filt[c], s=(512,512)),
               s=(512,512), norm="forward")[..., :256, :256] + x

DFTs as dense matmuls on the tensor engine, with the *data* always the
stationary operand (out = lhsT.T @ rhs flips the data layout each stage), so
the 4 contractions chain with zero transposes:

    s1: T[w,hf]  = sum_h  x[h,w]  * Ah[h,hf]        x:[H,W]   -> T:[W,HF]
    s2: S[hf,wf] = sum_w  T[w,hf] * Aw[w,wf]        T:[W,HF]  -> S:[HF,WF]
    oK: P = S * K[c]   (pointwise complex, DVE, fused with PSUM->SBUF)
    s3: V[wf,h]  = sum_hf P[hf,wf]* Bh[hf,h]        P:[HF,WF] -> V:[WF,H]
    s4: y[h,w]   = sum_wf Vre*Gc - Vim*Gs           V:[WF,H]  -> y:[H,W]
    y += x

Sharding: channels across the 8 cores (8 ch/core x 8 batch = 64 planes/core);
filter spectra K[c] computed once per core, cached in SBUF. All matmuls are
float32r (full-rate fp32, free dim >= 256).

TRN2 constraint: a fused fp32r matmul (S3_LW) can carry at most ONE sem wait.
Structure guarantees <=1 cross-engine dep per matmul:
  - per-stage PSUM pools so each slot's releasing engine is deterministic
    (s1: DVE T-copies; s2/s4 shared pool: DVE oK/residual; s3+nyq: ACT V-copies)
  - tiny "touch" matmuls absorb the DMA / producer dep into PE program order
    before each stage's first real matmul.
"""

import numpy as np
import ml_dtypes
from contextlib import ExitStack

import concourse.bass as bass
import concourse.mybir as mybir
import concourse.tile as tile
from concourse.bass_utils import run_bass_kernel_spmd

B, C, H, W = 8, 64, 256, 256
N = 512
HF = 512
WF = 257
WFP = 258          # even moving free size
NCORES = 8
CPC = C // NCORES
PLANES = CPC * B

F32 = mybir.dt.float32
F32R = mybir.dt.float32r
BF16 = mybir.dt.bfloat16
NPBF16 = ml_dtypes.bfloat16


def _consts():
    h = np.arange(H, dtype=np.float64)[:, None]
    hf = np.arange(HF, dtype=np.float64)[None, :]
    ah = np.exp(-2j * np.pi * h * hf / N)              # [256, 512]
    w = np.arange(W, dtype=np.float64)[:, None]
    wf = np.arange(WF, dtype=np.float64)[None, :]
    aw = np.exp(-2j * np.pi * w * wf / N)              # [256, 257]
    aw = np.concatenate([aw, np.zeros((W, 1))], axis=1)  # pad to 258 (even N)
    hf2 = np.arange(HF, dtype=np.float64)[:, None]
    h2 = np.arange(H, dtype=np.float64)[None, :]
    bh = np.exp(+2j * np.pi * hf2 * h2 / N)            # [512, 256]
    c = np.full((WF, 1), 2.0); c[0] = 1.0; c[256] = 1.0
    wf2 = np.arange(WF, dtype=np.float64)[:, None]
    w2 = np.arange(W, dtype=np.float64)[None, :]
    gc = c * np.cos(2 * np.pi * wf2 * w2 / N)          # [257, 256]
    gs = c * np.sin(2 * np.pi * wf2 * w2 / N)          # [257, 256]
    f = NPBF16
    # s3 pairs are fused into single 512-wide matmuls: rhs = [bhr|bhi] for
    # the lre operand and [-bhi|bhr] for the lim operand.
    bhri = np.concatenate([bh.real, bh.imag], axis=1)     # [512, 512]
    bhnr = np.concatenate([-bh.imag, bh.real], axis=1)    # [512, 512]
    d = {
        "ahr": (f(ah.real), 2), "ahi": (f(ah.imag), 2),
        "awr": (f(aw.real), 2), "awi": (f(aw.imag), 2), "awin": (f(-aw.imag), 2),
        "bhri": (f(bhri), 4), "bhnr": (f(bhnr), 4),
        "gc": (f(gc[:256]), 2), "gsn": (f(-gs[:256]), 2),
    }
    # one [128, F] blob in SBUF layout -> one DMA, one semaphore
    cols, offs, off = [], {}, 0
    for k, (arr, kt) in d.items():
        fd = arr.shape[1]
        cols.append(arr.reshape(kt, 128, fd).transpose(1, 0, 2).reshape(128, kt * fd))
        offs[k] = (off, fd)
        off += kt * fd
    # wf=256 (Nyquist) G row, replicated on all partitions for the
    # per-partition-scalar STT that applies it outside the PE.
    pm1 = np.tile(f(gc[256])[None, :], (128, 1))
    cols.append(pm1)
    offs["pm1"] = (off, W)
    return np.concatenate(cols, axis=1), offs


def _legalize_waits(nc, max_waits=1):
    """This walrus build allows only ONE sem wait per engine instruction
    ("Too many sync wait commands"). Split extra waits onto same-engine NOPs
    inserted immediately before — engine program order preserves semantics."""
    k = 0
    for fn in nc.m.functions:
        for bb in fn.blocks:
            new = []
            for ins in bb.instructions:
                si = ins.sync_info
                waits = list(si.on_wait) if (si and si.on_wait) else []
                if len(waits) > max_waits:
                    for w in waits[:-max_waits]:
                        k += 1
                        new.append(mybir.InstNoOp(
                            name=f"{ins.name}-lw{k}", engine=ins.engine,
                            ins=[], outs=[],
                            sync_info=mybir.SyncInfo(on_wait=[w], on_update=[])))
                    ins.sync_info = mybir.SyncInfo(
                        on_wait=waits[-max_waits:],
                        on_update=list(si.on_update or []))
                new.append(ins)
            bb.instructions = new
    return k


def _dedupe_ldweights(nc):
    """Skip the PE stationary reload when consecutive matmuls in the final
    engine order share the identical weights AP (verified on HW: a matmul
    with ldweights=False reuses the array contents left by the previous
    self-loading matmul)."""
    def sig(ins):
        w = ins.ins[1]
        mr = w.memref
        return (mr.name if hasattr(mr, "name") else str(mr),
                w.offset, str(w.ap), str(w.dtype),
                ins.is_transpose, str(ins.perf_mode),
                tuple(ins.tile_position or ()), tuple(ins.tile_size or ()))
    n = 0
    for fn in nc.m.functions:
        for bb in fn.blocks:
            prev = None
            for ins in bb.instructions:
                if not isinstance(ins, mybir.InstMatmult):
                    continue
                s = sig(ins)
                if prev is not None and s == prev:
                    ins.ldweights = False
                    n += 1
                prev = s
    return n


def build_nc(n_ch=CPC, n_b=B, reps=1):
    nc = bass.Bass(trn_type="TRN2")
    n_planes = n_ch * n_b

    xs = nc.dram_tensor("xs", [n_planes, H, W], BF16, kind="ExternalInput").ap()
    fs = nc.dram_tensor("fs", [n_ch, H, W], BF16, kind="ExternalInput").ap()
    cblob_np, coffs = _consts()
    cb_d = nc.dram_tensor("cblob", list(cblob_np.shape), BF16,
                          kind="ExternalInput").ap()
    ys = nc.dram_tensor("ys", [n_planes, H, W], F32, kind="ExternalOutput").ap()

    with tile.TileContext(nc) as tc, ExitStack() as ctx:
        const_p = ctx.enter_context(tc.tile_pool(name="const", bufs=1))
        kc_p = ctx.enter_context(tc.tile_pool(name="kc", bufs=1))
        x_p = ctx.enter_context(tc.tile_pool(name="xp", bufs=n_b + 2))
        t_p = ctx.enter_context(tc.tile_pool(name="tp", bufs=2))
        p_p = ctx.enter_context(tc.tile_pool(name="pp", bufs=2))
        v_p = ctx.enter_context(tc.tile_pool(name="vp", bufs=2))
        y_p = ctx.enter_context(tc.tile_pool(name="yp", bufs=4))
        tm_p = ctx.enter_context(tc.tile_pool(name="tm", bufs=8))
        ps1_p = ctx.enter_context(tc.tile_pool(name="ps1", bufs=2, space="PSUM"))
        psd_p = ctx.enter_context(tc.tile_pool(name="psd", bufs=3, space="PSUM"))
        ps3_p = ctx.enter_context(tc.tile_pool(name="ps3", bufs=2, space="PSUM"))
        dps_p = ctx.enter_context(tc.tile_pool(name="dps", bufs=1, space="PSUM"))

        cb = const_p.tile(list(cblob_np.shape), BF16, tag="cb")
        nc.sync.dma_start(out=cb, in_=cb_d)

        class CV:
            def __init__(self, name, fd):
                self.off, self.fd = coffs[name][0], fd
            def __getitem__(self, idx):
                p, k, fs_ = idx
                lo = self.off + k * self.fd
                if fs_ == slice(None):
                    return cb[p, lo:lo + self.fd]
                return cb[p, lo + fs_.start:lo + fs_.stop]

        ahr = CV("ahr", HF); ahi = CV("ahi", HF)
        awr = CV("awr", WFP); awi = CV("awi", WFP); awin = CV("awin", WFP)
        bhri = CV("bhri", 2 * H); bhnr = CV("bhnr", 2 * H)
        gc = CV("gc", W); gsn = CV("gsn", W)
        pm1r = cb[:, coffs["pm1"][0]:coffs["pm1"][0] + W]

        kre = kc_p.tile([128, n_ch, 4, WFP], BF16, tag="kre")
        kim = kc_p.tile([128, n_ch, 4, WFP], BF16, tag="kim")

        MM = nc.tensor.matmul
        # single dummy PSUM target for all "touch" matmuls (PE-only WAW)
        dps = dps_p.tile([1, 64], F32, tag="dps")

        def touch(src_ap):
            """Tiny matmul reading src so PE inherits its producer dep."""
            MM(dps, src_ap[0:1, 0:1], src_ap[0:1, 0:64], start=True, stop=True)

        # PE touches the const blob once; const deps then PE-dominated.
        touch(cb)

        def fwd(plane_ap, sink):
            """s1+s2 on one [256,256] DRAM plane; sink(mhf, sr, si) consumes
            the four [128,WF] PSUM spectrum chunk pairs. Returns x tile."""
            xt = x_p.tile([128, 2, W], BF16, tag="xt")
            nc.sync.dma_start(out=xt, in_=plane_ap.rearrange("(k p) w -> p k w", p=128))
            touch(xt[:, 0, :])          # absorb DMA wait
            tre = t_p.tile([128, 2, HF], BF16, tag="tre")
            tim = t_p.tile([128, 2, HF], BF16, tag="tim")
            for mw in range(2):
                pr = ps1_p.tile([128, HF], F32, tag="ps1")
                pi = ps1_p.tile([128, HF], F32, tag="ps1")
                for kh in range(2):
                    lhsT = xt[:, kh, mw * 128:(mw + 1) * 128]
                    MM(pr, lhsT, ahr[:, kh, :], start=(kh == 0), stop=(kh == 1))
                    MM(pi, lhsT, ahi[:, kh, :], start=(kh == 0), stop=(kh == 1))
                nc.scalar.copy(out=tre[:, mw, :], in_=pr)
                nc.scalar.copy(out=tim[:, mw, :], in_=pi)
            for mhf in range(4):
                sr = psd_p.tile([128, WFP], F32, tag="psd")
                si = psd_p.tile([128, WFP], F32, tag="psd")
                for kw in range(2):
                    lre = tre[:, kw, mhf * 128:(mhf + 1) * 128]
                    lim = tim[:, kw, mhf * 128:(mhf + 1) * 128]
                    MM(sr, lre, awr[:, kw, :], start=(kw == 0), stop=False)
                    MM(si, lre, awi[:, kw, :], start=(kw == 0), stop=False)
                    MM(sr, lim, awin[:, kw, :], start=False, stop=(kw == 1))
                    MM(si, lim, awr[:, kw, :], start=False, stop=(kw == 1))
                sink(mhf, sr, si)
            return xt

        # ---- filter spectra into K cache (DVE copies keep psd DVE-released)
        for ch in range(n_ch):
            def k_sink(mhf, sr, si, ch=ch):
                nc.vector.tensor_copy(kre[:, ch, mhf, :], sr)
                nc.vector.tensor_copy(kim[:, ch, mhf, :], si)
            fwd(fs[ch], k_sink)

        # ---- main plane loop (optionally repeated on-device for timing) ----
        rep_ctx = tc.For_i(0, reps, 1) if reps > 1 else None
        if rep_ctx is not None:
            rep_ctx.__enter__()
        MULT = mybir.AluOpType.mult
        ADD = mybir.AluOpType.add
        for ch in range(n_ch):
            # ---- wave: forward transforms + oK for all n_b planes of ch ----
            pre_all = p_p.tile([128, n_b, 4, WFP], BF16, tag="pre")
            pim_all = p_p.tile([128, n_b, 4, WFP], BF16, tag="pim")
            xts = []
            for b in range(n_b):
                def x_sink(mhf, sr, si, ch=ch, b=b):
                    krc = kre[:, ch, mhf, :]
                    kic = kim[:, ch, mhf, :]
                    t1 = tm_p.tile([128, WFP], F32, tag="tm")
                    t2 = tm_p.tile([128, WFP], F32, tag="tm")
                    t3 = tm_p.tile([128, WFP], F32, tag="tm")
                    t4 = tm_p.tile([128, WFP], F32, tag="tm")
                    nc.vector.tensor_mul(t1, sr, krc)
                    nc.vector.tensor_mul(t2, si, kic)
                    nc.vector.tensor_mul(t3, sr, kic)
                    nc.vector.tensor_mul(t4, si, krc)
                    nc.vector.tensor_sub(pre_all[:, b, mhf, :], t1, t2)
                    nc.vector.tensor_add(pim_all[:, b, mhf, :], t3, t4)

                xts.append(fwd(xs[ch * n_b + b], x_sink))

            # ---- batched Nyquist column (wf=256) for the whole wave:
            # pvnT[h, mh, b] = sum_hf P_b[hf, 256] * Bh[hf, h]; const
            # stationary, data moving (n_b-wide streams).
            pvnT = ps3_p.tile([128, 2, n_b], F32, tag="ps3")
            for mh in range(2):
                for khf in range(4):
                    MM(pvnT[:, mh, :], bhri[:, khf, mh * 128:(mh + 1) * 128],
                       pre_all[:, :, khf, 256], start=(khf == 0), stop=False)
                    MM(pvnT[:, mh, :], bhnr[:, khf, mh * 128:(mh + 1) * 128],
                       pim_all[:, :, khf, 256], start=False, stop=(khf == 3))
            vnyqT = v_p.tile([128, 2, n_b], BF16, tag="vnyqT")
            nc.scalar.copy(out=vnyqT, in_=pvnT)

            # ---- inverse transforms + output per plane ----
            for b in range(n_b):
                pl = ch * n_b + b
                touch(pre_all[:, b, 0, :])   # absorb DVE oK dep before s3
                v_both = v_p.tile([128, 2, 2 * H], BF16, tag="vb")
                for mwf in range(2):
                    pvb = ps3_p.tile([128, 2 * H], F32, tag="ps3")
                    for khf in range(4):
                        lre = pre_all[:, b, khf, mwf * 128:(mwf + 1) * 128]
                        lim = pim_all[:, b, khf, mwf * 128:(mwf + 1) * 128]
                        MM(pvb, lre, bhri[:, khf, :], start=(khf == 0), stop=False)
                        MM(pvb, lim, bhnr[:, khf, :], start=False, stop=(khf == 3))
                    nc.scalar.copy(out=v_both[:, mwf, :], in_=pvb)

                touch(v_both[:, 0, 0:256])  # absorb ACT V-copy dep before s4
                ysb = y_p.tile([128, 2, W], F32, tag="ysb")
                xt = xts[b]
                for mh in range(2):
                    py = psd_p.tile([128, W], F32, tag="psd")
                    MM(py, v_both[:, 0, mh * 128:(mh + 1) * 128], gc[:, 0, :],
                       start=True, stop=False)
                    MM(py, v_both[:, 0, 256 + mh * 128:256 + (mh + 1) * 128],
                       gsn[:, 0, :], start=False, stop=False)
                    MM(py, v_both[:, 1, mh * 128:(mh + 1) * 128], gc[:, 1, :],
                       start=False, stop=False)
                    MM(py, v_both[:, 1, 256 + mh * 128:256 + (mh + 1) * 128],
                       gsn[:, 1, :], start=False, stop=True)
                    # Nyquist term + residual: tny = pm1r * vnyq[:,mh,b] + x
                    tny = tm_p.tile([128, W], F32, tag="tm")
                    nc.vector.scalar_tensor_tensor(
                        tny, pm1r, vnyqT[:, mh, b:b + 1], xt[:, mh, :],
                        MULT, ADD)
                    nc.vector.tensor_add(ysb[:, mh, :], tny, py)
                nc.sync.dma_start(out=ys[pl].rearrange("(k p) w -> p k w", p=128),
                                  in_=ysb)
        if rep_ctx is not None:
            rep_ctx.__exit__(None, None, None)
    _dedupe_ldweights(nc)
    _legalize_waits(nc)
    return nc


def kernel(x: np.ndarray, filt: np.ndarray) -> np.ndarray:
    x = np.ascontiguousarray(x, dtype=np.float32)
    xb = x.astype(NPBF16)
    fb = np.ascontiguousarray(filt, dtype=np.float32).astype(NPBF16)
    cblob = _consts()[0]
    nc = build_nc()
    in_maps = []
    for i in range(NCORES):
        sl = slice(i * CPC, (i + 1) * CPC)
        xsh = np.ascontiguousarray(
            xb[:, sl].transpose(1, 0, 2, 3).reshape(PLANES, H, W))
        in_maps.append({"xs": xsh, "fs": np.ascontiguousarray(fb[sl]),
                        "cblob": cblob})
    res = run_bass_kernel_spmd(nc, in_maps, core_ids=list(range(NCORES)))
    out = np.empty_like(x)
    for i in range(NCORES):
        sl = slice(i * CPC, (i + 1) * CPC)
        out[:, sl] = res.results[i]["ys"].reshape(CPC, B, H, W).transpose(1, 0, 2, 3)
    return out



# revision 22
# speedup vs baseline: 1.6369x; 1.2305x over previous
"""FFT-based 2D long convolution on 8 Trainium2 NeuronCores.

Reference op (per (b,c) plane, 512x512 FFT):
    y = irfft2(rfft2(x, s=(512,512)) * rfft2(filt[c], s=(512,512)),
               s=(512,512), norm="forward")[..., :256, :256] + x

DFTs as dense matmuls on the tensor engine, with the *data* always the
stationary operand (out = lhsT.T @ rhs flips the data layout each stage), so
the 4 contractions chain with zero transposes:

    s1: T[w,hf]  = sum_h  x[h,w]  * Ah[h,hf]        x:[H,W]   -> T:[W,HF]
    s2: S[hf,wf] = sum_w  T[w,hf] * Aw[w,wf]        T:[W,HF]  -> S:[HF,WF]
    oK: P = S * K[c]   (pointwise complex, DVE, fused with PSUM->SBUF)
    s3: V[wf,h]  = sum_hf P[hf,wf]* Bh[hf,h]        P:[HF,WF] -> V:[WF,H]
    s4: y[h,w]   = sum_wf Vre*Gc - Vim*Gs           V:[WF,H]  -> y:[H,W]
    y += x

Sharding: channels across the 8 cores (8 ch/core x 8 batch = 64 planes/core);
filter spectra K[c] computed once per core, cached in SBUF. All matmuls are
float32r (full-rate fp32, free dim >= 256).

TRN2 constraint: a fused fp32r matmul (S3_LW) can carry at most ONE sem wait.
Structure guarantees <=1 cross-engine dep per matmul:
  - per-stage PSUM pools so each slot's releasing engine is deterministic
    (s1: DVE T-copies; s2/s4 shared pool: DVE oK/residual; s3+nyq: ACT V-copies)
  - tiny "touch" matmuls absorb the DMA / producer dep into PE program order
    before each stage's first real matmul.
"""

import numpy as np
import ml_dtypes
from contextlib import ExitStack

import concourse.bass as bass
import concourse.mybir as mybir
import concourse.tile as tile
from concourse.bass_utils import run_bass_kernel_spmd

B, C, H, W = 8, 64, 256, 256
N = 512
HF = 512
WF = 257
WFP = 258          # even moving free size
NCORES = 8
CPC = C // NCORES
PLANES = CPC * B

F32 = mybir.dt.float32
F32R = mybir.dt.float32r
BF16 = mybir.dt.bfloat16
NPBF16 = ml_dtypes.bfloat16


def _consts():
    h = np.arange(H, dtype=np.float64)[:, None]
    hf = np.arange(HF, dtype=np.float64)[None, :]
    ah = np.exp(-2j * np.pi * h * hf / N)              # [256, 512]
    w = np.arange(W, dtype=np.float64)[:, None]
    wf = np.arange(WF, dtype=np.float64)[None, :]
    aw = np.exp(-2j * np.pi * w * wf / N)              # [256, 257]
    aw = np.concatenate([aw, np.zeros((W, 1))], axis=1)  # pad to 258 (even N)
    hf2 = np.arange(HF, dtype=np.float64)[:, None]
    h2 = np.arange(H, dtype=np.float64)[None, :]
    bh = np.exp(+2j * np.pi * hf2 * h2 / N)            # [512, 256]
    c = np.full((WF, 1), 2.0); c[0] = 1.0; c[256] = 1.0
    wf2 = np.arange(WF, dtype=np.float64)[:, None]
    w2 = np.arange(W, dtype=np.float64)[None, :]
    gc = c * np.cos(2 * np.pi * wf2 * w2 / N)          # [257, 256]
    gs = c * np.sin(2 * np.pi * wf2 * w2 / N)          # [257, 256]
    f = NPBF16
    # s3 pairs are fused into single 512-wide matmuls: rhs = [bhr|bhi] for
    # the lre operand and [-bhi|bhr] for the lim operand.
    bhri = np.concatenate([bh.real, bh.imag], axis=1)     # [512, 512]
    bhnr = np.concatenate([-bh.imag, bh.real], axis=1)    # [512, 512]
    d = {
        "ahr": (f(ah.real), 2), "ahi": (f(ah.imag), 2),
        "awr": (f(aw.real), 2), "awi": (f(aw.imag), 2), "awin": (f(-aw.imag), 2),
        "bhri": (f(bhri), 4), "bhnr": (f(bhnr), 4),
        "gc": (f(gc[:256]), 2), "gsn": (f(-gs[:256]), 2),
    }
    # one [128, F] blob in SBUF layout -> one DMA, one semaphore
    cols, offs, off = [], {}, 0
    for k, (arr, kt) in d.items():
        fd = arr.shape[1]
        cols.append(arr.reshape(kt, 128, fd).transpose(1, 0, 2).reshape(128, kt * fd))
        offs[k] = (off, fd)
        off += kt * fd
    # wf=256 (Nyquist) G row, replicated on all partitions for the
    # per-partition-scalar STT that applies it outside the PE.
    pm1 = np.tile(f(gc[256])[None, :], (128, 1))
    cols.append(pm1)
    offs["pm1"] = (off, W)
    return np.concatenate(cols, axis=1), offs


def _legalize_waits(nc, max_waits=1):
    """This walrus build allows only ONE sem wait per engine instruction
    ("Too many sync wait commands"). Split extra waits onto same-engine NOPs
    inserted immediately before — engine program order preserves semantics."""
    k = 0
    for fn in nc.m.functions:
        for bb in fn.blocks:
            new = []
            for ins in bb.instructions:
                si = ins.sync_info
                waits = list(si.on_wait) if (si and si.on_wait) else []
                if len(waits) > max_waits:
                    for w in waits[:-max_waits]:
                        k += 1
                        new.append(mybir.InstNoOp(
                            name=f"{ins.name}-lw{k}", engine=ins.engine,
                            ins=[], outs=[],
                            sync_info=mybir.SyncInfo(on_wait=[w], on_update=[])))
                    ins.sync_info = mybir.SyncInfo(
                        on_wait=waits[-max_waits:],
                        on_update=list(si.on_update or []))
                new.append(ins)
            bb.instructions = new
    return k


def _dedupe_ldweights(nc):
    """Skip the PE stationary reload when consecutive matmuls in the final
    engine order share the identical weights AP (verified on HW: a matmul
    with ldweights=False reuses the array contents left by the previous
    self-loading matmul)."""
    def sig(ins):
        w = ins.ins[1]
        mr = w.memref
        return (mr.name if hasattr(mr, "name") else str(mr),
                w.offset, str(w.ap), str(w.dtype),
                ins.is_transpose, str(ins.perf_mode),
                tuple(ins.tile_position or ()), tuple(ins.tile_size or ()))
    n = 0
    for fn in nc.m.functions:
        for bb in fn.blocks:
            prev = None
            for ins in bb.instructions:
                if not isinstance(ins, mybir.InstMatmult):
                    continue
                s = sig(ins)
                if prev is not None and s == prev:
                    ins.ldweights = False
                    n += 1
                prev = s
    return n


def build_nc(n_ch=CPC, n_b=B, reps=1):
    nc = bass.Bass(trn_type="TRN2")
    n_planes = n_ch * n_b

    xs = nc.dram_tensor("xs", [n_planes, H, W], BF16, kind="ExternalInput").ap()
    fs = nc.dram_tensor("fs", [n_ch, H, W], BF16, kind="ExternalInput").ap()
    cblob_np, coffs = _consts()
    cb_d = nc.dram_tensor("cblob", list(cblob_np.shape), BF16,
                          kind="ExternalInput").ap()
    ys = nc.dram_tensor("ys", [n_planes, H, W], F32, kind="ExternalOutput").ap()

    with tile.TileContext(nc) as tc, ExitStack() as ctx:
        const_p = ctx.enter_context(tc.tile_pool(name="const", bufs=1))
        kc_p = ctx.enter_context(tc.tile_pool(name="kc", bufs=1))
        x_p = ctx.enter_context(tc.tile_pool(name="xp", bufs=n_b + 2))
        t_p = ctx.enter_context(tc.tile_pool(name="tp", bufs=2))
        p_p = ctx.enter_context(tc.tile_pool(name="pp", bufs=2))
        v_p = ctx.enter_context(tc.tile_pool(name="vp", bufs=2))
        y_p = ctx.enter_context(tc.tile_pool(name="yp", bufs=4))
        tm_p = ctx.enter_context(tc.tile_pool(name="tm", bufs=16))
        ps1_p = ctx.enter_context(tc.tile_pool(name="ps1", bufs=2, space="PSUM"))
        psd_p = ctx.enter_context(tc.tile_pool(name="psd", bufs=3, space="PSUM"))
        ps3_p = ctx.enter_context(tc.tile_pool(name="ps3", bufs=2, space="PSUM"))
        dps_p = ctx.enter_context(tc.tile_pool(name="dps", bufs=1, space="PSUM"))

        cb = const_p.tile(list(cblob_np.shape), BF16, tag="cb")
        nc.sync.dma_start(out=cb, in_=cb_d)

        class CV:
            def __init__(self, name, fd):
                self.off, self.fd = coffs[name][0], fd
            def __getitem__(self, idx):
                p, k, fs_ = idx
                lo = self.off + k * self.fd
                if fs_ == slice(None):
                    return cb[p, lo:lo + self.fd]
                return cb[p, lo + fs_.start:lo + fs_.stop]

        ahr = CV("ahr", HF); ahi = CV("ahi", HF)
        awr = CV("awr", WFP); awi = CV("awi", WFP); awin = CV("awin", WFP)
        bhri = CV("bhri", 2 * H); bhnr = CV("bhnr", 2 * H)
        gc = CV("gc", W); gsn = CV("gsn", W)
        pm1r = cb[:, coffs["pm1"][0]:coffs["pm1"][0] + W]

        kre = kc_p.tile([128, n_ch, 4, WFP], BF16, tag="kre")
        kim = kc_p.tile([128, n_ch, 4, WFP], BF16, tag="kim")

        MM = nc.tensor.matmul
        # single dummy PSUM target for all "touch" matmuls (PE-only WAW)
        dps = dps_p.tile([1, 64], F32, tag="dps")

        def touch(src_ap):
            """Tiny matmul reading src so PE inherits its producer dep."""
            MM(dps, src_ap[0:1, 0:1], src_ap[0:1, 0:64], start=True, stop=True)

        # PE touches the const blob once; const deps then PE-dominated.
        touch(cb)

        def fwd(plane_ap, sink):
            """s1+s2 on one [256,256] DRAM plane; sink(mhf, sr, si) consumes
            the four [128,WF] PSUM spectrum chunk pairs. Returns x tile."""
            xt = x_p.tile([128, 2, W], BF16, tag="xt")
            nc.sync.dma_start(out=xt, in_=plane_ap.rearrange("(k p) w -> p k w", p=128))
            touch(xt[:, 0, :])          # absorb DMA wait
            tre = t_p.tile([128, 2, HF], BF16, tag="tre")
            tim = t_p.tile([128, 2, HF], BF16, tag="tim")
            for mw in range(2):
                pr = ps1_p.tile([128, HF], F32, tag="ps1")
                pi = ps1_p.tile([128, HF], F32, tag="ps1")
                for kh in range(2):
                    lhsT = xt[:, kh, mw * 128:(mw + 1) * 128]
                    MM(pr, lhsT, ahr[:, kh, :], start=(kh == 0), stop=(kh == 1))
                    MM(pi, lhsT, ahi[:, kh, :], start=(kh == 0), stop=(kh == 1))
                nc.scalar.copy(out=tre[:, mw, :], in_=pr)
                nc.scalar.copy(out=tim[:, mw, :], in_=pi)
            for mhf in range(4):
                sr = psd_p.tile([128, WFP], F32, tag="psd")
                si = psd_p.tile([128, WFP], F32, tag="psd")
                for kw in range(2):
                    lre = tre[:, kw, mhf * 128:(mhf + 1) * 128]
                    lim = tim[:, kw, mhf * 128:(mhf + 1) * 128]
                    MM(sr, lre, awr[:, kw, :], start=(kw == 0), stop=False)
                    MM(si, lre, awi[:, kw, :], start=(kw == 0), stop=False)
                    MM(sr, lim, awin[:, kw, :], start=False, stop=(kw == 1))
                    MM(si, lim, awr[:, kw, :], start=False, stop=(kw == 1))
                sink(mhf, sr, si)
            return xt

        # ---- filter spectra into K cache (DVE copies keep psd DVE-released)
        for ch in range(n_ch):
            def k_sink(mhf, sr, si, ch=ch):
                nc.vector.tensor_copy(kre[:, ch, mhf, :], sr)
                nc.vector.tensor_copy(kim[:, ch, mhf, :], si)
            fwd(fs[ch], k_sink)

        # ---- main plane loop (optionally repeated on-device for timing) ----
        rep_ctx = tc.For_i(0, reps, 1) if reps > 1 else None
        if rep_ctx is not None:
            rep_ctx.__enter__()
        MULT = mybir.AluOpType.mult
        ADD = mybir.AluOpType.add
        for ch in range(n_ch):
            # ---- wave: forward transforms + oK for all n_b planes of ch ----
            pre_all = p_p.tile([128, n_b, 4, WFP], BF16, tag="pre")
            pim_all = p_p.tile([128, n_b, 4, WFP], BF16, tag="pim")
            xts = []
            for b in range(n_b):
                def x_sink(mhf, sr, si, ch=ch, b=b):
                    krc = kre[:, ch, mhf, :]
                    kic = kim[:, ch, mhf, :]
                    # bounce PSUM->SBUF bf16 on the scalar engine so all six
                    # DVE ops below run at 16-bit (2x) rate
                    sbr = tm_p.tile([128, WFP], BF16, tag="tm")
                    sbi = tm_p.tile([128, WFP], BF16, tag="tm")
                    nc.scalar.copy(out=sbr, in_=sr)
                    nc.scalar.copy(out=sbi, in_=si)
                    t1 = tm_p.tile([128, WFP], BF16, tag="tm")
                    t2 = tm_p.tile([128, WFP], BF16, tag="tm")
                    t3 = tm_p.tile([128, WFP], BF16, tag="tm")
                    t4 = tm_p.tile([128, WFP], BF16, tag="tm")
                    nc.vector.tensor_mul(t1, sbr, krc)
                    nc.vector.tensor_mul(t2, sbi, kic)
                    nc.vector.tensor_mul(t3, sbr, kic)
                    nc.vector.tensor_mul(t4, sbi, krc)
                    nc.vector.tensor_sub(pre_all[:, b, mhf, :], t1, t2)
                    nc.vector.tensor_add(pim_all[:, b, mhf, :], t3, t4)

                xts.append(fwd(xs[ch * n_b + b], x_sink))

            # ---- batched Nyquist column (wf=256) for the whole wave:
            # pvnT[h, mh, b] = sum_hf P_b[hf, 256] * Bh[hf, h]; const
            # stationary, data moving (n_b-wide streams).
            pvnT = ps3_p.tile([128, 2, n_b], F32, tag="ps3")
            for mh in range(2):
                for khf in range(4):
                    MM(pvnT[:, mh, :], bhri[:, khf, mh * 128:(mh + 1) * 128],
                       pre_all[:, :, khf, 256], start=(khf == 0), stop=False)
                    MM(pvnT[:, mh, :], bhnr[:, khf, mh * 128:(mh + 1) * 128],
                       pim_all[:, :, khf, 256], start=False, stop=(khf == 3))
            vnyqT = v_p.tile([128, 2, n_b], BF16, tag="vnyqT")
            nc.scalar.copy(out=vnyqT, in_=pvnT)

            # ---- inverse transforms + output per plane ----
            for b in range(n_b):
                pl = ch * n_b + b
                touch(pre_all[:, b, 0, :])   # absorb DVE oK dep before s3
                v_both = v_p.tile([128, 2, 2 * H], BF16, tag="vb")
                for mwf in range(2):
                    pvb = ps3_p.tile([128, 2 * H], F32, tag="ps3")
                    for khf in range(4):
                        lre = pre_all[:, b, khf, mwf * 128:(mwf + 1) * 128]
                        lim = pim_all[:, b, khf, mwf * 128:(mwf + 1) * 128]
                        MM(pvb, lre, bhri[:, khf, :], start=(khf == 0), stop=False)
                        MM(pvb, lim, bhnr[:, khf, :], start=False, stop=(khf == 3))
                    nc.scalar.copy(out=v_both[:, mwf, :], in_=pvb)

                touch(v_both[:, 0, 0:256])  # absorb ACT V-copy dep before s4
                ysb = y_p.tile([128, 2, W], F32, tag="ysb")
                xt = xts[b]
                for mh in range(2):
                    py = psd_p.tile([128, W], F32, tag="psd")
                    MM(py, v_both[:, 0, mh * 128:(mh + 1) * 128], gc[:, 0, :],
                       start=True, stop=False)
                    MM(py, v_both[:, 0, 256 + mh * 128:256 + (mh + 1) * 128],
                       gsn[:, 0, :], start=False, stop=False)
                    MM(py, v_both[:, 1, mh * 128:(mh + 1) * 128], gc[:, 1, :],
                       start=False, stop=False)
                    MM(py, v_both[:, 1, 256 + mh * 128:256 + (mh + 1) * 128],
                       gsn[:, 1, :], start=False, stop=True)
                    # Nyquist term + residual: tny = pm1r * vnyq[:,mh,b] + x
                    tny = tm_p.tile([128, W], F32, tag="tm")
                    nc.vector.scalar_tensor_tensor(
                        tny, pm1r, vnyqT[:, mh, b:b + 1], xt[:, mh, :],
                        MULT, ADD)
                    nc.vector.tensor_add(ysb[:, mh, :], tny, py)
                nc.sync.dma_start(out=ys[pl].rearrange("(k p) w -> p k w", p=128),
                                  in_=ysb)
        if rep_ctx is not None:
            rep_ctx.__exit__(None, None, None)
    _dedupe_ldweights(nc)
    _legalize_waits(nc)
    return nc


def kernel(x: np.ndarray, filt: np.ndarray) -> np.ndarray:
    x = np.ascontiguousarray(x, dtype=np.float32)
    xb = x.astype(NPBF16)
    fb = np.ascontiguousarray(filt, dtype=np.float32).astype(NPBF16)
    cblob = _consts()[0]
    nc = build_nc()
    in_maps = []
    for i in range(NCORES):
        sl = slice(i * CPC, (i + 1) * CPC)
        xsh = np.ascontiguousarray(
            xb[:, sl].transpose(1, 0, 2, 3).reshape(PLANES, H, W))
        in_maps.append({"xs": xsh, "fs": np.ascontiguousarray(fb[sl]),
                        "cblob": cblob})
    res = run_bass_kernel_spmd(nc, in_maps, core_ids=list(range(NCORES)))
    out = np.empty_like(x)
    for i in range(NCORES):
        sl = slice(i * CPC, (i + 1) * CPC)
        out[:, sl] = res.results[i]["ys"].reshape(CPC, B, H, W).transpose(1, 0, 2, 3)
    return out



# revision 31
# speedup vs baseline: 1.7720x; 1.0825x over previous
"""FFT-based 2D long convolution on 8 Trainium2 NeuronCores.

Reference op (per (b,c) plane, 512x512 FFT):
    y = irfft2(rfft2(x, s=(512,512)) * rfft2(filt[c], s=(512,512)),
               s=(512,512), norm="forward")[..., :256, :256] + x

DFTs as dense matmuls on the tensor engine, with the *data* always the
stationary operand (out = lhsT.T @ rhs flips the data layout each stage), so
the 4 contractions chain with zero transposes:

    s1: T[w,hf]  = sum_h  x[h,w]  * Ah[h,hf]        x:[H,W]   -> T:[W,HF]
    s2: S[hf,wf] = sum_w  T[w,hf] * Aw[w,wf]        T:[W,HF]  -> S:[HF,WF]
    oK: P = S * K[c]   (pointwise complex, DVE, fused with PSUM->SBUF)
    s3: V[wf,h]  = sum_hf P[hf,wf]* Bh[hf,h]        P:[HF,WF] -> V:[WF,H]
    s4: y[h,w]   = sum_wf Vre*Gc - Vim*Gs           V:[WF,H]  -> y:[H,W]
    y += x

Sharding: channels across the 8 cores (8 ch/core x 8 batch = 64 planes/core);
filter spectra K[c] computed once per core, cached in SBUF. All matmuls are
float32r (full-rate fp32, free dim >= 256).

TRN2 constraint: a fused fp32r matmul (S3_LW) can carry at most ONE sem wait.
Structure guarantees <=1 cross-engine dep per matmul:
  - per-stage PSUM pools so each slot's releasing engine is deterministic
    (s1: DVE T-copies; s2/s4 shared pool: DVE oK/residual; s3+nyq: ACT V-copies)
  - tiny "touch" matmuls absorb the DMA / producer dep into PE program order
    before each stage's first real matmul.
"""

import numpy as np
import ml_dtypes
from contextlib import ExitStack

import concourse.bass as bass
import concourse.mybir as mybir
import concourse.tile as tile
from concourse.bass_utils import run_bass_kernel_spmd

B, C, H, W = 8, 64, 256, 256
N = 512
HF = 512
WF = 257
WFP = 258          # even moving free size
NCORES = 8
CPC = C // NCORES
PLANES = CPC * B

F32 = mybir.dt.float32
F32R = mybir.dt.float32r
BF16 = mybir.dt.bfloat16
NPBF16 = ml_dtypes.bfloat16


def _hfidx():
    """hf row order of the four spectrum chunks: A0, A1 direct; B0', B1'
    hold conj(S~[g]) rows at hf=512-g (so every hf appears exactly once)."""
    return np.concatenate([
        np.arange(0, 128), np.arange(128, 256),
        np.arange(511, 383, -1), np.arange(383, 255, -1)])


def _consts():
    h = np.arange(H, dtype=np.float64)[:, None]
    hf = np.arange(WFP, dtype=np.float64)[None, :]     # only g in [0,257]
    ah = np.exp(-2j * np.pi * h * hf / N)              # [256, 258]
    w = np.arange(W, dtype=np.float64)[:, None]
    wf = np.arange(WF, dtype=np.float64)[None, :]
    aw = np.exp(-2j * np.pi * w * wf / N)              # [256, 257]
    aw = np.concatenate([aw, np.zeros((W, 1))], axis=1)  # pad to 258 (even N)
    hf2 = _hfidx()[:, None].astype(np.float64)
    h2 = np.arange(H, dtype=np.float64)[None, :]
    bh = np.exp(+2j * np.pi * hf2 * h2 / N)            # [512, 256], rows permuted
    c = np.full((WF, 1), 2.0); c[0] = 1.0; c[256] = 1.0
    wf2 = np.arange(WF, dtype=np.float64)[:, None]
    w2 = np.arange(W, dtype=np.float64)[None, :]
    gc = c * np.cos(2 * np.pi * wf2 * w2 / N)          # [257, 256]
    gs = c * np.sin(2 * np.pi * wf2 * w2 / N)          # [257, 256]
    f = NPBF16
    # s3 pairs are fused into single 512-wide matmuls: rhs = [bhr|bhi] for
    # the lre operand and [-bhi|bhr] for the lim operand.
    bhri = np.concatenate([bh.real, bh.imag], axis=1)     # [512, 512]
    bhnr = np.concatenate([-bh.imag, bh.real], axis=1)    # [512, 512]
    d = {
        "ahr": (f(ah.real), 2), "ahi": (f(ah.imag), 2),
        "awr": (f(aw.real), 2), "awi": (f(aw.imag), 2), "awin": (f(-aw.imag), 2),
        "bhri": (f(bhri), 4), "bhnr": (f(bhnr), 4),
        "gc": (f(gc[:256]), 2), "gsn": (f(-gs[:256]), 2),
    }
    # one [128, F] blob in SBUF layout -> one DMA, one semaphore
    cols, offs, off = [], {}, 0
    for k, (arr, kt) in d.items():
        fd = arr.shape[1]
        cols.append(arr.reshape(kt, 128, fd).transpose(1, 0, 2).reshape(128, kt * fd))
        offs[k] = (off, fd)
        off += kt * fd
    # wf=256 (Nyquist) G row, replicated on all partitions for the
    # per-partition-scalar STT that applies it outside the PE.
    pm1 = np.tile(f(gc[256])[None, :], (128, 1))
    cols.append(pm1)
    offs["pm1"] = (off, W)
    return np.concatenate(cols, axis=1), offs


def _legalize_waits(nc, max_waits=1):
    """This walrus build allows only ONE sem wait per engine instruction
    ("Too many sync wait commands"). Split extra waits onto same-engine NOPs
    inserted immediately before — engine program order preserves semantics."""
    k = 0
    for fn in nc.m.functions:
        for bb in fn.blocks:
            new = []
            for ins in bb.instructions:
                si = ins.sync_info
                waits = list(si.on_wait) if (si and si.on_wait) else []
                if len(waits) > max_waits:
                    for w in waits[:-max_waits]:
                        k += 1
                        new.append(mybir.InstNoOp(
                            name=f"{ins.name}-lw{k}", engine=ins.engine,
                            ins=[], outs=[],
                            sync_info=mybir.SyncInfo(on_wait=[w], on_update=[])))
                    ins.sync_info = mybir.SyncInfo(
                        on_wait=waits[-max_waits:],
                        on_update=list(si.on_update or []))
                new.append(ins)
            bb.instructions = new
    return k


def _dedupe_ldweights(nc):
    """Skip the PE stationary reload when consecutive matmuls in the final
    engine order share the identical weights AP (verified on HW: a matmul
    with ldweights=False reuses the array contents left by the previous
    self-loading matmul)."""
    def sig(ins):
        w = ins.ins[1]
        mr = w.memref
        return (mr.name if hasattr(mr, "name") else str(mr),
                w.offset, str(w.ap), str(w.dtype),
                ins.is_transpose, str(ins.perf_mode),
                tuple(ins.tile_position or ()), tuple(ins.tile_size or ()))
    n = 0
    for fn in nc.m.functions:
        for bb in fn.blocks:
            prev = None
            for ins in bb.instructions:
                if not isinstance(ins, mybir.InstMatmult):
                    continue
                s = sig(ins)
                if prev is not None and s == prev:
                    ins.ldweights = False
                    n += 1
                prev = s
    return n


def build_nc(n_ch=CPC, n_b=B, reps=1):
    nc = bass.Bass(trn_type="TRN2")
    n_planes = n_ch * n_b

    xs = nc.dram_tensor("xs", [n_planes, H, W], BF16, kind="ExternalInput").ap()
    kr_d = nc.dram_tensor("kr", [128, n_ch, 4, WFP], BF16,
                          kind="ExternalInput").ap()
    ki_d = nc.dram_tensor("ki", [128, n_ch, 4, WFP], BF16,
                          kind="ExternalInput").ap()
    cblob_np, coffs = _consts()
    cb_d = nc.dram_tensor("cblob", list(cblob_np.shape), BF16,
                          kind="ExternalInput").ap()
    ys = nc.dram_tensor("ys", [n_planes, H, W], F32, kind="ExternalOutput").ap()

    with tile.TileContext(nc) as tc, ExitStack() as ctx:
        const_p = ctx.enter_context(tc.tile_pool(name="const", bufs=1))
        kc_p = ctx.enter_context(tc.tile_pool(name="kc", bufs=1))
        x_p = ctx.enter_context(tc.tile_pool(name="xp", bufs=n_b + 2))
        t_p = ctx.enter_context(tc.tile_pool(name="tp", bufs=2))
        p_p = ctx.enter_context(tc.tile_pool(name="pp", bufs=2))
        v_p = ctx.enter_context(tc.tile_pool(name="vp", bufs=2))
        y_p = ctx.enter_context(tc.tile_pool(name="yp", bufs=4))
        tm_p = ctx.enter_context(tc.tile_pool(name="tm", bufs=16))
        ps1_p = ctx.enter_context(tc.tile_pool(name="ps1", bufs=2, space="PSUM"))
        psd_p = ctx.enter_context(tc.tile_pool(name="psd", bufs=3, space="PSUM"))
        ps3_p = ctx.enter_context(tc.tile_pool(name="ps3", bufs=2, space="PSUM"))
        dps_p = ctx.enter_context(tc.tile_pool(name="dps", bufs=1, space="PSUM"))

        cb = const_p.tile(list(cblob_np.shape), BF16, tag="cb")
        nc.sync.dma_start(out=cb, in_=cb_d)

        class CV:
            def __init__(self, name, fd):
                self.off, self.fd = coffs[name][0], fd
            def __getitem__(self, idx):
                p, k, fs_ = idx
                lo = self.off + k * self.fd
                if fs_ == slice(None):
                    return cb[p, lo:lo + self.fd]
                return cb[p, lo + fs_.start:lo + fs_.stop]

        ahr = CV("ahr", WFP); ahi = CV("ahi", WFP)
        awr = CV("awr", WFP); awi = CV("awi", WFP); awin = CV("awin", WFP)
        bhri = CV("bhri", 2 * H); bhnr = CV("bhnr", 2 * H)
        gc = CV("gc", W); gsn = CV("gsn", W)
        pm1r = cb[:, coffs["pm1"][0]:coffs["pm1"][0] + W]

        kre = kc_p.tile([128, n_ch, 4, WFP], BF16, tag="kre")
        kim = kc_p.tile([128, n_ch, 4, WFP], BF16, tag="kim")
        nc.sync.dma_start(out=kre, in_=kr_d)
        nc.sync.dma_start(out=kim, in_=ki_d)

        MM = nc.tensor.matmul
        # single dummy PSUM target for all "touch" matmuls (PE-only WAW)
        dps = dps_p.tile([1, 64], F32, tag="dps")

        def touch(src_ap):
            """Tiny matmul reading src so PE inherits its producer dep."""
            MM(dps, src_ap[0:1, 0:1], src_ap[0:1, 0:64], start=True, stop=True)

        # PE touches the const blob once; const deps then PE-dominated.
        touch(cb)

        # s2 output chunks: A0, A1 are the direct spectrum rows hf=g for
        # g in [0,256); B0', B1' are S~[g] = sum_w T[w,g]*conj(Aw[w,wf])
        # whose conjugate supplies rows hf=512-g (K/Bh rows are permuted
        # on the host to match, see _hfidx).
        CHUNK_G0 = (0, 128, 1, 129)      # lhsT column start per chunk
        CHUNK_CONJ = (False, False, True, True)

        def fwd(plane_ap, sink):
            """s1+s2 on one [256,256] DRAM plane; sink(mhf, sr, si) consumes
            the four [128,WF] PSUM spectrum chunk pairs. Returns x tile."""
            xt = x_p.tile([128, 2, W], BF16, tag="xt")
            nc.sync.dma_start(out=xt, in_=plane_ap.rearrange("(k p) w -> p k w", p=128))
            touch(xt[:, 0, :])          # absorb DMA wait
            tre = t_p.tile([128, 2, WFP], BF16, tag="tre")
            tim = t_p.tile([128, 2, WFP], BF16, tag="tim")
            for mw in range(2):
                pr = ps1_p.tile([128, WFP], F32, tag="ps1")
                pi = ps1_p.tile([128, WFP], F32, tag="ps1")
                for kh in range(2):
                    lhsT = xt[:, kh, mw * 128:(mw + 1) * 128]
                    MM(pr, lhsT, ahr[:, kh, :], start=(kh == 0), stop=(kh == 1))
                    MM(pi, lhsT, ahi[:, kh, :], start=(kh == 0), stop=(kh == 1))
                nc.scalar.copy(out=tre[:, mw, :], in_=pr)
                nc.scalar.copy(out=tim[:, mw, :], in_=pi)
            for mhf in range(4):
                g0 = CHUNK_G0[mhf]
                cj = CHUNK_CONJ[mhf]
                sr = psd_p.tile([128, WFP], F32, tag="psd")
                si = psd_p.tile([128, WFP], F32, tag="psd")
                for kw in range(2):
                    lre = tre[:, kw, g0:g0 + 128]
                    lim = tim[:, kw, g0:g0 + 128]
                    MM(sr, lre, awr[:, kw, :], start=(kw == 0), stop=False)
                    MM(si, lre, (awin if cj else awi)[:, kw, :],
                       start=(kw == 0), stop=False)
                    MM(sr, lim, (awi if cj else awin)[:, kw, :],
                       start=False, stop=(kw == 1))
                    MM(si, lim, awr[:, kw, :], start=False, stop=(kw == 1))
                sink(mhf, sr, si)
            return xt

        # ---- main plane loop (optionally repeated on-device for timing) ----
        rep_ctx = tc.For_i(0, reps, 1) if reps > 1 else None
        if rep_ctx is not None:
            rep_ctx.__enter__()
        MULT = mybir.AluOpType.mult
        ADD = mybir.AluOpType.add
        for ch in range(n_ch):
            # ---- wave: forward transforms + oK for all n_b planes of ch ----
            pre_all = p_p.tile([128, n_b, 4, WFP], BF16, tag="pre")
            pim_all = p_p.tile([128, n_b, 4, WFP], BF16, tag="pim")
            xts = []
            for b in range(n_b):
                def x_sink(mhf, sr, si, ch=ch, b=b):
                    krc = kre[:, ch, mhf, :]
                    kic = kim[:, ch, mhf, :]
                    # bounce PSUM->SBUF bf16 on the scalar engine so all six
                    # DVE ops below run at 16-bit (2x) rate
                    sbr = tm_p.tile([128, WFP], BF16, tag="tm")
                    sbi = tm_p.tile([128, WFP], BF16, tag="tm")
                    nc.scalar.copy(out=sbr, in_=sr)
                    nc.scalar.copy(out=sbi, in_=si)
                    t1 = tm_p.tile([128, WFP], BF16, tag="tm")
                    t2 = tm_p.tile([128, WFP], BF16, tag="tm")
                    t3 = tm_p.tile([128, WFP], BF16, tag="tm")
                    t4 = tm_p.tile([128, WFP], BF16, tag="tm")
                    nc.vector.tensor_mul(t1, sbr, krc)
                    nc.vector.tensor_mul(t2, sbi, kic)
                    nc.vector.tensor_mul(t3, sbr, kic)
                    nc.vector.tensor_mul(t4, sbi, krc)
                    if CHUNK_CONJ[mhf]:
                        # chunk rows hold conj(S~); P = conj(S~) * K
                        nc.vector.tensor_add(pre_all[:, b, mhf, :], t1, t2)
                        nc.vector.tensor_sub(pim_all[:, b, mhf, :], t3, t4)
                    else:
                        nc.vector.tensor_sub(pre_all[:, b, mhf, :], t1, t2)
                        nc.vector.tensor_add(pim_all[:, b, mhf, :], t3, t4)

                xts.append(fwd(xs[ch * n_b + b], x_sink))

            # ---- batched Nyquist column (wf=256) for the whole wave:
            # pvnT[h, mh, b] = sum_hf P_b[hf, 256] * Bh[hf, h]; const
            # stationary, data moving (n_b-wide streams).
            pvnT = ps3_p.tile([128, 2, n_b], F32, tag="ps3")
            for mh in range(2):
                for khf in range(4):
                    MM(pvnT[:, mh, :], bhri[:, khf, mh * 128:(mh + 1) * 128],
                       pre_all[:, :, khf, 256], start=(khf == 0), stop=False)
                    MM(pvnT[:, mh, :], bhnr[:, khf, mh * 128:(mh + 1) * 128],
                       pim_all[:, :, khf, 256], start=False, stop=(khf == 3))
            vnyqT = v_p.tile([128, 2, n_b], BF16, tag="vnyqT")
            nc.scalar.copy(out=vnyqT, in_=pvnT)

            # ---- inverse transforms + output per plane ----
            for b in range(n_b):
                pl = ch * n_b + b
                touch(pre_all[:, b, 0, :])   # absorb DVE oK dep before s3
                v_both = v_p.tile([128, 2, 2 * H], BF16, tag="vb")
                for mwf in range(2):
                    pvb = ps3_p.tile([128, 2 * H], F32, tag="ps3")
                    for khf in range(4):
                        lre = pre_all[:, b, khf, mwf * 128:(mwf + 1) * 128]
                        lim = pim_all[:, b, khf, mwf * 128:(mwf + 1) * 128]
                        MM(pvb, lre, bhri[:, khf, :], start=(khf == 0), stop=False)
                        MM(pvb, lim, bhnr[:, khf, :], start=False, stop=(khf == 3))
                    nc.scalar.copy(out=v_both[:, mwf, :], in_=pvb)

                touch(v_both[:, 0, 0:256])  # absorb ACT V-copy dep before s4
                ysb = y_p.tile([128, 2, W], F32, tag="ysb")
                xt = xts[b]
                for mh in range(2):
                    py = psd_p.tile([128, W], F32, tag="psd")
                    MM(py, v_both[:, 0, mh * 128:(mh + 1) * 128], gc[:, 0, :],
                       start=True, stop=False)
                    MM(py, v_both[:, 0, 256 + mh * 128:256 + (mh + 1) * 128],
                       gsn[:, 0, :], start=False, stop=False)
                    MM(py, v_both[:, 1, mh * 128:(mh + 1) * 128], gc[:, 1, :],
                       start=False, stop=False)
                    MM(py, v_both[:, 1, 256 + mh * 128:256 + (mh + 1) * 128],
                       gsn[:, 1, :], start=False, stop=True)
                    # Nyquist term + residual: tny = pm1r * vnyq[:,mh,b] + x
                    tny = tm_p.tile([128, W], F32, tag="tm")
                    nc.vector.scalar_tensor_tensor(
                        tny, pm1r, vnyqT[:, mh, b:b + 1], xt[:, mh, :],
                        MULT, ADD)
                    nc.vector.tensor_add(ysb[:, mh, :], tny, py)
                nc.sync.dma_start(out=ys[pl].rearrange("(k p) w -> p k w", p=128),
                                  in_=ysb)
        if rep_ctx is not None:
            rep_ctx.__exit__(None, None, None)
    _dedupe_ldweights(nc)
    _legalize_waits(nc)
    return nc


def filter_spectra(filt_slice: np.ndarray):
    """Host-side rfft2 of the filter channels -> K-cache layout
    [128(hf within chunk), n_ch, 4(hf chunk), WFP] bf16 (re, im)."""
    n_ch = filt_slice.shape[0]
    kf = np.fft.rfft2(filt_slice.astype(np.float64), s=(N, N))  # [n_ch,512,257]
    kf = kf[:, _hfidx(), :]                                    # chunk row order
    kk = kf.reshape(n_ch, 4, 128, WF).transpose(2, 0, 1, 3)    # [128,n_ch,4,257]
    out = np.zeros((2, 128, n_ch, 4, WFP), np.float32)
    out[0, :, :, :, :WF] = kk.real
    out[1, :, :, :, :WF] = kk.imag
    return out[0].astype(NPBF16), out[1].astype(NPBF16)


def kernel(x: np.ndarray, filt: np.ndarray) -> np.ndarray:
    x = np.ascontiguousarray(x, dtype=np.float32)
    xb = x.astype(NPBF16)
    filt = np.ascontiguousarray(filt, dtype=np.float32)
    cblob = _consts()[0]
    nc = build_nc()
    in_maps = []
    for i in range(NCORES):
        sl = slice(i * CPC, (i + 1) * CPC)
        xsh = np.ascontiguousarray(
            xb[:, sl].transpose(1, 0, 2, 3).reshape(PLANES, H, W))
        kr, ki = filter_spectra(filt[sl])
        in_maps.append({"xs": xsh, "kr": kr, "ki": ki,
                        "cblob": cblob})
    res = run_bass_kernel_spmd(nc, in_maps, core_ids=list(range(NCORES)))
    out = np.empty_like(x)
    for i in range(NCORES):
        sl = slice(i * CPC, (i + 1) * CPC)
        out[:, sl] = res.results[i]["ys"].reshape(CPC, B, H, W).transpose(1, 0, 2, 3)
    return out



# revision 35
# speedup vs baseline: 1.8201x; 1.0271x over previous
"""FFT-based 2D long convolution on 8 Trainium2 NeuronCores.

Reference op (per (b,c) plane, 512x512 FFT):
    y = irfft2(rfft2(x, s=(512,512)) * rfft2(filt[c], s=(512,512)),
               s=(512,512), norm="forward")[..., :256, :256] + x

DFTs as dense matmuls on the tensor engine, with the *data* always the
stationary operand (out = lhsT.T @ rhs flips the data layout each stage), so
the 4 contractions chain with zero transposes:

    s1: T[w,hf]  = sum_h  x[h,w]  * Ah[h,hf]        x:[H,W]   -> T:[W,HF]
    s2: S[hf,wf] = sum_w  T[w,hf] * Aw[w,wf]        T:[W,HF]  -> S:[HF,WF]
    oK: P = S * K[c]   (pointwise complex, DVE, fused with PSUM->SBUF)
    s3: V[wf,h]  = sum_hf P[hf,wf]* Bh[hf,h]        P:[HF,WF] -> V:[WF,H]
    s4: y[h,w]   = sum_wf Vre*Gc - Vim*Gs           V:[WF,H]  -> y:[H,W]
    y += x

Sharding: channels across the 8 cores (8 ch/core x 8 batch = 64 planes/core);
filter spectra K[c] computed once per core, cached in SBUF. All matmuls are
float32r (full-rate fp32, free dim >= 256).

TRN2 constraint: a fused fp32r matmul (S3_LW) can carry at most ONE sem wait.
Structure guarantees <=1 cross-engine dep per matmul:
  - per-stage PSUM pools so each slot's releasing engine is deterministic
    (s1: DVE T-copies; s2/s4 shared pool: DVE oK/residual; s3+nyq: ACT V-copies)
  - tiny "touch" matmuls absorb the DMA / producer dep into PE program order
    before each stage's first real matmul.
"""

import numpy as np
import ml_dtypes
from contextlib import ExitStack

import concourse.bass as bass
import concourse.mybir as mybir
import concourse.tile as tile
from concourse.bass_utils import run_bass_kernel_spmd

B, C, H, W = 8, 64, 256, 256
N = 512
HF = 512
WF = 257
WFP = 258          # even moving free size
NCORES = 8
CPC = C // NCORES
PLANES = CPC * B

F32 = mybir.dt.float32
F32R = mybir.dt.float32r
BF16 = mybir.dt.bfloat16
NPBF16 = ml_dtypes.bfloat16


def _hfidx():
    """hf row order of the four spectrum chunks: A0, A1 direct; B0', B1'
    hold conj(S~[g]) rows at hf=512-g (so every hf appears exactly once)."""
    return np.concatenate([
        np.arange(0, 128), np.arange(128, 256),
        np.arange(511, 383, -1), np.arange(383, 255, -1)])


def _consts():
    h = np.arange(H, dtype=np.float64)[:, None]
    hf = np.arange(WFP, dtype=np.float64)[None, :]     # only g in [0,257]
    ah = np.exp(-2j * np.pi * h * hf / N)              # [256, 258]
    w = np.arange(W, dtype=np.float64)[:, None]
    wf = np.arange(WF, dtype=np.float64)[None, :]
    aw = np.exp(-2j * np.pi * w * wf / N)              # [256, 257]
    aw = np.concatenate([aw, np.zeros((W, 1))], axis=1)  # pad to 258 (even N)
    hf2 = _hfidx()[:, None].astype(np.float64)
    h2 = np.arange(H, dtype=np.float64)[None, :]
    bh = np.exp(+2j * np.pi * hf2 * h2 / N)            # [512, 256], rows permuted
    c = np.full((WF, 1), 2.0); c[0] = 1.0; c[256] = 1.0
    wf2 = np.arange(WF, dtype=np.float64)[:, None]
    w2 = np.arange(W, dtype=np.float64)[None, :]
    gc = c * np.cos(2 * np.pi * wf2 * w2 / N)          # [257, 256]
    gs = c * np.sin(2 * np.pi * wf2 * w2 / N)          # [257, 256]
    f = NPBF16
    # s3 pairs are fused into single 512-wide matmuls: rhs = [bhr|bhi] for
    # the lre operand and [-bhi|bhr] for the lim operand.
    bhri = np.concatenate([bh.real, bh.imag], axis=1)     # [512, 512]
    bhnr = np.concatenate([-bh.imag, bh.real], axis=1)    # [512, 512]
    d = {
        "ahr": (f(ah.real), 2), "ahi": (f(ah.imag), 2),
        "awr": (f(aw.real), 2), "awi": (f(aw.imag), 2), "awin": (f(-aw.imag), 2),
        "bhri": (f(bhri), 4), "bhnr": (f(bhnr), 4),
        "gc": (f(gc[:256]), 2), "gsn": (f(-gs[:256]), 2),
    }
    # one [128, F] blob in SBUF layout -> one DMA, one semaphore
    cols, offs, off = [], {}, 0
    for k, (arr, kt) in d.items():
        fd = arr.shape[1]
        cols.append(arr.reshape(kt, 128, fd).transpose(1, 0, 2).reshape(128, kt * fd))
        offs[k] = (off, fd)
        off += kt * fd
    # wf=256 (Nyquist) G row, replicated on all partitions for the
    # per-partition-scalar STT that applies it outside the PE.
    pm1 = np.tile(f(gc[256])[None, :], (128, 1))
    cols.append(pm1)
    offs["pm1"] = (off, W)
    return np.concatenate(cols, axis=1), offs


def _legalize_waits(nc, max_waits=1):
    """This walrus build allows only ONE sem wait per engine instruction
    ("Too many sync wait commands"). Split extra waits onto same-engine NOPs
    inserted immediately before — engine program order preserves semantics."""
    k = 0
    for fn in nc.m.functions:
        for bb in fn.blocks:
            new = []
            for ins in bb.instructions:
                si = ins.sync_info
                waits = list(si.on_wait) if (si and si.on_wait) else []
                if len(waits) > max_waits:
                    for w in waits[:-max_waits]:
                        k += 1
                        new.append(mybir.InstNoOp(
                            name=f"{ins.name}-lw{k}", engine=ins.engine,
                            ins=[], outs=[],
                            sync_info=mybir.SyncInfo(on_wait=[w], on_update=[])))
                    ins.sync_info = mybir.SyncInfo(
                        on_wait=waits[-max_waits:],
                        on_update=list(si.on_update or []))
                new.append(ins)
            bb.instructions = new
    return k


def _dedupe_ldweights(nc):
    """Skip the PE stationary reload when consecutive matmuls in the final
    engine order share the identical weights AP (verified on HW: a matmul
    with ldweights=False reuses the array contents left by the previous
    self-loading matmul)."""
    def sig(ins):
        w = ins.ins[1]
        mr = w.memref
        return (mr.name if hasattr(mr, "name") else str(mr),
                w.offset, str(w.ap), str(w.dtype),
                ins.is_transpose, str(ins.perf_mode),
                tuple(ins.tile_position or ()), tuple(ins.tile_size or ()))
    n = 0
    for fn in nc.m.functions:
        for bb in fn.blocks:
            prev = None
            for ins in bb.instructions:
                if not isinstance(ins, mybir.InstMatmult):
                    continue
                s = sig(ins)
                if prev is not None and s == prev:
                    ins.ldweights = False
                    n += 1
                prev = s
    return n


def build_nc(n_ch=CPC, n_b=B, reps=1):
    nc = bass.Bass(trn_type="TRN2")
    n_planes = n_ch * n_b

    xs = nc.dram_tensor("xs", [n_planes, H, W], BF16, kind="ExternalInput").ap()
    kr_d = nc.dram_tensor("kr", [128, n_ch, 4, WFP], BF16,
                          kind="ExternalInput").ap()
    ki_d = nc.dram_tensor("ki", [128, n_ch, 4, WFP], BF16,
                          kind="ExternalInput").ap()
    cblob_np, coffs = _consts()
    cb_d = nc.dram_tensor("cblob", list(cblob_np.shape), BF16,
                          kind="ExternalInput").ap()
    ys = nc.dram_tensor("ys", [n_planes, H, W], F32, kind="ExternalOutput").ap()

    with tile.TileContext(nc) as tc, ExitStack() as ctx:
        const_p = ctx.enter_context(tc.tile_pool(name="const", bufs=1))
        kc_p = ctx.enter_context(tc.tile_pool(name="kc", bufs=1))
        x_p = ctx.enter_context(tc.tile_pool(name="xp", bufs=n_b + 2))
        t_p = ctx.enter_context(tc.tile_pool(name="tp", bufs=2))
        p_p = ctx.enter_context(tc.tile_pool(name="pp", bufs=2))
        v_p = ctx.enter_context(tc.tile_pool(name="vp", bufs=2))
        y_p = ctx.enter_context(tc.tile_pool(name="yp", bufs=4))
        tm_p = ctx.enter_context(tc.tile_pool(name="tm", bufs=16))
        ps1_p = ctx.enter_context(tc.tile_pool(name="ps1", bufs=2, space="PSUM"))
        psd_p = ctx.enter_context(tc.tile_pool(name="psd", bufs=3, space="PSUM"))
        ps3_p = ctx.enter_context(tc.tile_pool(name="ps3", bufs=2, space="PSUM"))
        dps_p = ctx.enter_context(tc.tile_pool(name="dps", bufs=1, space="PSUM"))

        cb = const_p.tile(list(cblob_np.shape), BF16, tag="cb")
        nc.sync.dma_start(out=cb, in_=cb_d)

        class CV:
            def __init__(self, name, fd):
                self.off, self.fd = coffs[name][0], fd
            def __getitem__(self, idx):
                p, k, fs_ = idx
                lo = self.off + k * self.fd
                if fs_ == slice(None):
                    return cb[p, lo:lo + self.fd]
                return cb[p, lo + fs_.start:lo + fs_.stop]

        ahr = CV("ahr", WFP); ahi = CV("ahi", WFP)
        awr = CV("awr", WFP); awi = CV("awi", WFP); awin = CV("awin", WFP)
        bhri = CV("bhri", 2 * H); bhnr = CV("bhnr", 2 * H)
        gc = CV("gc", W); gsn = CV("gsn", W)
        pm1r = cb[:, coffs["pm1"][0]:coffs["pm1"][0] + W]

        kre = kc_p.tile([128, n_ch, 4, WFP], BF16, tag="kre")
        kim = kc_p.tile([128, n_ch, 4, WFP], BF16, tag="kim")
        nc.sync.dma_start(out=kre, in_=kr_d)
        nc.sync.dma_start(out=kim, in_=ki_d)

        MM = nc.tensor.matmul
        # single dummy PSUM target for all "touch" matmuls (PE-only WAW)
        dps = dps_p.tile([1, 64], F32, tag="dps")

        def touch(src_ap):
            """Tiny matmul reading src so PE inherits its producer dep."""
            MM(dps, src_ap[0:1, 0:1], src_ap[0:1, 0:64], start=True, stop=True)

        # PE touches the const blob once; const deps then PE-dominated.
        touch(cb)

        # s2 output chunks: A0, A1 are the direct spectrum rows hf=g for
        # g in [0,256); B0', B1' are S~[g] = sum_w T[w,g]*conj(Aw[w,wf])
        # whose conjugate supplies rows hf=512-g (K/Bh rows are permuted
        # on the host to match, see _hfidx).
        CHUNK_G0 = (0, 128, 1, 129)      # lhsT column start per chunk
        CHUNK_CONJ = (False, False, True, True)

        def fwd(plane_ap, ch, b, pre_all, pim_all):
            """s1+s2+oK on one [256,256] DRAM plane; spectrum chunk pairs are
            multiplied by the filter spectrum with 516-wide bf16 DVE ops and
            written into pre_all/pim_all[:, b]. Returns x tile."""
            xt = x_p.tile([128, 2, W], BF16, tag="xt")
            nc.sync.dma_start(out=xt, in_=plane_ap.rearrange("(k p) w -> p k w", p=128))
            touch(xt[:, 0, :])          # absorb DMA wait
            tre = t_p.tile([128, 2, WFP], BF16, tag="tre")
            tim = t_p.tile([128, 2, WFP], BF16, tag="tim")
            for mw in range(2):
                pr = ps1_p.tile([128, WFP], F32, tag="ps1")
                pi = ps1_p.tile([128, WFP], F32, tag="ps1")
                for kh in range(2):
                    lhsT = xt[:, kh, mw * 128:(mw + 1) * 128]
                    MM(pr, lhsT, ahr[:, kh, :], start=(kh == 0), stop=(kh == 1))
                    MM(pi, lhsT, ahi[:, kh, :], start=(kh == 0), stop=(kh == 1))
                nc.scalar.copy(out=tre[:, mw, :], in_=pr)
                nc.scalar.copy(out=tim[:, mw, :], in_=pi)
            for mp in range(2):         # chunk pairs (0,1) and (2,3)
                sbr2 = tm_p.tile([128, 2, WFP], BF16, tag="tm")
                sbi2 = tm_p.tile([128, 2, WFP], BF16, tag="tm")
                for m2 in range(2):
                    mhf = 2 * mp + m2
                    g0 = CHUNK_G0[mhf]
                    cj = CHUNK_CONJ[mhf]
                    sr = psd_p.tile([128, WFP], F32, tag="psd")
                    si = psd_p.tile([128, WFP], F32, tag="psd")
                    for kw in range(2):
                        lre = tre[:, kw, g0:g0 + 128]
                        lim = tim[:, kw, g0:g0 + 128]
                        MM(sr, lre, awr[:, kw, :], start=(kw == 0), stop=False)
                        MM(si, lre, (awin if cj else awi)[:, kw, :],
                           start=(kw == 0), stop=False)
                        MM(sr, lim, (awi if cj else awin)[:, kw, :],
                           start=False, stop=(kw == 1))
                        MM(si, lim, awr[:, kw, :], start=False, stop=(kw == 1))
                    # bounce PSUM->SBUF bf16 (scalar engine) so the DVE ops
                    # below run at 16-bit rate on the whole pair at once
                    nc.scalar.copy(out=sbr2[:, m2, :], in_=sr)
                    nc.scalar.copy(out=sbi2[:, m2, :], in_=si)
                kpr = kre[:, ch, 2 * mp:2 * mp + 2, :]
                kpi = kim[:, ch, 2 * mp:2 * mp + 2, :]
                t1 = tm_p.tile([128, 2, WFP], BF16, tag="tm")
                t2 = tm_p.tile([128, 2, WFP], BF16, tag="tm")
                t3 = tm_p.tile([128, 2, WFP], BF16, tag="tm")
                t4 = tm_p.tile([128, 2, WFP], BF16, tag="tm")
                nc.vector.tensor_mul(t1, sbr2, kpr)
                nc.vector.tensor_mul(t2, sbi2, kpi)
                nc.vector.tensor_mul(t3, sbr2, kpi)
                nc.vector.tensor_mul(t4, sbi2, kpr)
                po = pre_all[:, b, 2 * mp:2 * mp + 2, :]
                qo = pim_all[:, b, 2 * mp:2 * mp + 2, :]
                if mp == 1:             # rows hold conj(S~); P = conj(S~)*K
                    nc.vector.tensor_add(po, t1, t2)
                    nc.vector.tensor_sub(qo, t3, t4)
                else:
                    nc.vector.tensor_sub(po, t1, t2)
                    nc.vector.tensor_add(qo, t3, t4)
            return xt

        # ---- main plane loop (optionally repeated on-device for timing) ----
        rep_ctx = tc.For_i(0, reps, 1) if reps > 1 else None
        if rep_ctx is not None:
            rep_ctx.__enter__()
        MULT = mybir.AluOpType.mult
        ADD = mybir.AluOpType.add
        for ch in range(n_ch):
            # ---- wave: forward transforms + oK for all n_b planes of ch ----
            pre_all = p_p.tile([128, n_b, 4, WFP], BF16, tag="pre")
            pim_all = p_p.tile([128, n_b, 4, WFP], BF16, tag="pim")
            xts = []
            for b in range(n_b):
                xts.append(fwd(xs[ch * n_b + b], ch, b, pre_all, pim_all))

            # ---- batched Nyquist column (wf=256) for the whole wave:
            # pvnT[h, mh, b] = sum_hf P_b[hf, 256] * Bh[hf, h]; const
            # stationary, data moving (n_b-wide streams).
            pvnT = ps3_p.tile([128, 2, n_b], F32, tag="ps3")
            for mh in range(2):
                for khf in range(4):
                    MM(pvnT[:, mh, :], bhri[:, khf, mh * 128:(mh + 1) * 128],
                       pre_all[:, :, khf, 256], start=(khf == 0), stop=False)
                    MM(pvnT[:, mh, :], bhnr[:, khf, mh * 128:(mh + 1) * 128],
                       pim_all[:, :, khf, 256], start=False, stop=(khf == 3))
            vnyqT = v_p.tile([128, 2, n_b], BF16, tag="vnyqT")
            nc.scalar.copy(out=vnyqT, in_=pvnT)

            # ---- inverse transforms + output per plane ----
            for b in range(n_b):
                pl = ch * n_b + b
                touch(pre_all[:, b, 0, :])   # absorb DVE oK dep before s3
                v_both = v_p.tile([128, 2, 2 * H], BF16, tag="vb")
                for mwf in range(2):
                    pvb = ps3_p.tile([128, 2 * H], F32, tag="ps3")
                    for khf in range(4):
                        lre = pre_all[:, b, khf, mwf * 128:(mwf + 1) * 128]
                        lim = pim_all[:, b, khf, mwf * 128:(mwf + 1) * 128]
                        MM(pvb, lre, bhri[:, khf, :], start=(khf == 0), stop=False)
                        MM(pvb, lim, bhnr[:, khf, :], start=False, stop=(khf == 3))
                    nc.scalar.copy(out=v_both[:, mwf, :], in_=pvb)

                touch(v_both[:, 0, 0:256])  # absorb ACT V-copy dep before s4
                ysb = y_p.tile([128, 2, W], F32, tag="ysb")
                xt = xts[b]
                for mh in range(2):
                    py = psd_p.tile([128, W], F32, tag="psd")
                    MM(py, v_both[:, 0, mh * 128:(mh + 1) * 128], gc[:, 0, :],
                       start=True, stop=False)
                    MM(py, v_both[:, 0, 256 + mh * 128:256 + (mh + 1) * 128],
                       gsn[:, 0, :], start=False, stop=False)
                    MM(py, v_both[:, 1, mh * 128:(mh + 1) * 128], gc[:, 1, :],
                       start=False, stop=False)
                    MM(py, v_both[:, 1, 256 + mh * 128:256 + (mh + 1) * 128],
                       gsn[:, 1, :], start=False, stop=True)
                    # Nyquist term + residual: tny = pm1r * vnyq[:,mh,b] + x
                    tny = tm_p.tile([128, W], F32, tag="tm")
                    nc.vector.scalar_tensor_tensor(
                        tny, pm1r, vnyqT[:, mh, b:b + 1], xt[:, mh, :],
                        MULT, ADD)
                    nc.vector.tensor_add(ysb[:, mh, :], tny, py)
                nc.sync.dma_start(out=ys[pl].rearrange("(k p) w -> p k w", p=128),
                                  in_=ysb)
        if rep_ctx is not None:
            rep_ctx.__exit__(None, None, None)
    _dedupe_ldweights(nc)
    _legalize_waits(nc)
    return nc


def filter_spectra(filt_slice: np.ndarray):
    """Host-side rfft2 of the filter channels -> K-cache layout
    [128(hf within chunk), n_ch, 4(hf chunk), WFP] bf16 (re, im)."""
    n_ch = filt_slice.shape[0]
    kf = np.fft.rfft2(filt_slice.astype(np.float64), s=(N, N))  # [n_ch,512,257]
    kf = kf[:, _hfidx(), :]                                    # chunk row order
    kk = kf.reshape(n_ch, 4, 128, WF).transpose(2, 0, 1, 3)    # [128,n_ch,4,257]
    out = np.zeros((2, 128, n_ch, 4, WFP), np.float32)
    out[0, :, :, :, :WF] = kk.real
    out[1, :, :, :, :WF] = kk.imag
    return out[0].astype(NPBF16), out[1].astype(NPBF16)


def kernel(x: np.ndarray, filt: np.ndarray) -> np.ndarray:
    x = np.ascontiguousarray(x, dtype=np.float32)
    xb = x.astype(NPBF16)
    filt = np.ascontiguousarray(filt, dtype=np.float32)
    cblob = _consts()[0]
    nc = build_nc()
    in_maps = []
    for i in range(NCORES):
        sl = slice(i * CPC, (i + 1) * CPC)
        xsh = np.ascontiguousarray(
            xb[:, sl].transpose(1, 0, 2, 3).reshape(PLANES, H, W))
        kr, ki = filter_spectra(filt[sl])
        in_maps.append({"xs": xsh, "kr": kr, "ki": ki,
                        "cblob": cblob})
    res = run_bass_kernel_spmd(nc, in_maps, core_ids=list(range(NCORES)))
    out = np.empty_like(x)
    for i in range(NCORES):
        sl = slice(i * CPC, (i + 1) * CPC)
        out[:, sl] = res.results[i]["ys"].reshape(CPC, B, H, W).transpose(1, 0, 2, 3)
    return out

